# revision 1
# baseline (speedup 1.0000x reference)
"""AttnDecoderBlock on 8 TRN2 NeuronCores — data-parallel over batch.

Per batch b:
  k   = LN_E(key[b]) ; kp = einsum('me,ec->cm', k, Wk)
  q   = LN_C(query[b])
  att = softmax(q @ kp, axis=-1)
  out = att @ (value[b] @ Wv)

Sharding: batch 64 -> 8 cores x 8 batches each. Weights/params replicated.

Per-core pipeline (row-major orientation, query rows on partitions):
  - query row r of a batch lives at partition r//32, tile r%32, so every DMA
    moves 16KB contiguous per partition (max DMA efficiency).
  - LN stats: per-tile bn_stats + batched even/odd Chan combine.
  - rsqrt(var) as Exp(-0.5*Ln(var+eps)) so the only ACT table set used is
    natural_log_exp_and_others (Ln/Exp/Copy/Identity) — no table thrash.
  - LN apply on GPSIMD (tensor_scalar), freeing DVE/ACT.
  - PE transposes q_ln tiles; ACT copies PSUM->SBUF casting to fp32r.
  - scores^T = Kx^T @ qlnT in fp32r (full rate at 512 moving columns).
  - softmax exp on ACT, bias = beta2-fold t_m - 15 (shift cancels in the
    normalize), output fp16 (fits after shift; 8x finer mantissa than bf16).
  - out tile = E_t @ [Vproj | 1] in fp16; denominator lands as column 128.
  - normalize on DVE: reciprocal + broadcast tensor_tensor from PSUM.
LN gamma/beta folds: g1 into Wk, b1 into tk column, g2 into Kx, b2 into the
exp bias t_m. value path has no LN.
"""

import numpy as np

import concourse.bass as bass
import concourse.mybir as mybir
import concourse.tile as tile
from concourse.bass_utils import run_bass_kernel_spmd
from concourse.masks import make_identity

B, N, M, E, C = 64, 4096, 64, 256, 128
NCORES = 8
BPC = B // NCORES          # batches per core
TQ = N // 128              # 32 row-tiles per batch
QG = TQ // 4               # 8 quads per batch
EPS = 1e-5
EXP_SHIFT = -15.0          # softmax shift; cancels in normalization
F32 = mybir.dt.float32
F16 = mybir.dt.float16

_ctr = [0]


def _split_waits(nc, limit=1):
    """The axon-path walrus accepts only `limit` sem-waits per instruction;
    move excess onto preceding same-engine NOPs (program order on the engine
    makes this equivalent)."""
    for f in nc.m.functions:
        for bb in f.blocks:
            out = []
            changed = False
            for inst in bb.instructions:
                si = inst.sync_info
                if si is not None and si.on_wait is not None and len(si.on_wait) > limit:
                    waits = list(si.on_wait)
                    while len(waits) > limit:
                        chunk, waits = waits[:limit], waits[limit:]
                        _ctr[0] += 1
                        nop = mybir.InstNoOp(name=f"I-wsplit-{_ctr[0]}", ins=[], outs=[])
                        nop.engine = inst.engine
                        nop.sync_info = mybir.SyncInfo(on_wait=chunk, on_update=[])
                        out.append(nop)
                        changed = True
                    inst.sync_info = mybir.SyncInfo(on_wait=waits, on_update=si.on_update)
                out.append(inst)
            if changed:
                bb.instructions = out
    return nc


def _build_nc(split=True):
    nc = bass.Bass()
    AF = mybir.ActivationFunctionType
    ALU = mybir.AluOpType

    query = nc.dram_tensor("query", [BPC, N, C], F32, kind="ExternalInput")
    key = nc.dram_tensor("key", [BPC, M, E], F32, kind="ExternalInput")
    value = nc.dram_tensor("value", [BPC, M, E], F32, kind="ExternalInput")
    wk = nc.dram_tensor("wk", [E, C], F32, kind="ExternalInput")
    wv = nc.dram_tensor("wv", [E, C], F32, kind="ExternalInput")
    g1 = nc.dram_tensor("g1", [E], F32, kind="ExternalInput")
    b1 = nc.dram_tensor("b1", [E], F32, kind="ExternalInput")
    g2 = nc.dram_tensor("g2", [C], F32, kind="ExternalInput")
    b2 = nc.dram_tensor("b2", [C], F32, kind="ExternalInput")
    out = nc.dram_tensor("out", [BPC, N, C], F32, kind="ExternalOutput")

    import contextlib
    with tile.TileContext(nc) as tc, contextlib.ExitStack() as ctx:
        singles = ctx.enter_context(tc.tile_pool(name="singles", bufs=1))

        ident = singles.tile([128, 128], F32, tag="ident")
        make_identity(nc, ident)
        ident16 = singles.tile([128, 128], F16, tag="ident16")
        nc.gpsimd.tensor_copy(ident16, ident)
        eps_col = singles.tile([128, 1], F32, tag="eps")
        nc.vector.memset(eps_col, EPS)

        # ---- weights / params ----
        wk_sb = singles.tile([128, 2, C], F32, tag="wk")       # [p, h, c], e=128h+p
        wv_sb = singles.tile([128, 2, C], F32, tag="wv")
        nc.sync.dma_start(out=wk_sb, in_=wk.rearrange("(h p) c -> p h c", p=128))
        nc.sync.dma_start(out=wv_sb, in_=wv.rearrange("(h p) c -> p h c", p=128))
        g1c = singles.tile([128, 2], F32, tag="g1")
        b1c = singles.tile([128, 2], F32, tag="b1")
        nc.sync.dma_start(out=g1c, in_=g1.rearrange("(h p) -> p h", p=128))
        nc.sync.dma_start(out=b1c, in_=b1.rearrange("(h p) -> p h", p=128))
        g2c = singles.tile([128, 1], F32, tag="g2")
        b2c = singles.tile([128, 1], F32, tag="b2")
        nc.sync.dma_start(out=g2c, in_=g2[:].unsqueeze(1))
        nc.sync.dma_start(out=b2c, in_=b2[:].unsqueeze(1))

        kx_all = singles.tile([128, BPC, M], F16, tag="kx")     # g2 (.) KP
        tcol_all = singles.tile([128, BPC], F32, tag="tcol")    # t_m + shift, dup rows
        v2_all = singles.tile([128, BPC, C + 1], F16, tag="v2") # [Vproj | 1], dup rows

        # ---- main loop ----
        qpool = ctx.enter_context(tc.tile_pool(name="qpool", bufs=3))
        opool = ctx.enter_context(tc.tile_pool(name="opool", bufs=3))
        small = ctx.enter_context(tc.tile_pool(name="small", bufs=3))
        mid = ctx.enter_context(tc.tile_pool(name="mid", bufs=4))

        state = {}
        H = TQ // 2

        def phase_a(j, hb):
            if hb == 0:
                q_sb = qpool.tile([128, TQ, C], F32, tag="q")
                qst = small.tile([128, TQ, 6], F32, tag="qst")
                mu = small.tile([128, TQ], F32, tag="mu")
                qr = small.tile([128, TQ], F32, tag="qr")
                dlt = small.tile([128, TQ], F32, tag="dlt")
                v128 = small.tile([128, TQ], F32, tag="v128")
                qlv = small.tile([128, TQ], F32, tag="qlv")
                negmur = small.tile([128, TQ], F32, tag="negmur")
                qln = qpool.tile([128, TQ, C], F16, tag="qln")
                o_sb = opool.tile([128, TQ, C], F32, tag="o")
                state[j] = (q_sb, qst, mu, qr, dlt, v128, qlv, negmur, qln, o_sb)
            q_sb, qst, mu, qr, dlt, v128, qlv, negmur, qln, o_sb = state[j]
            qsrc = query[j].rearrange("(p t) c -> p t c", t=TQ)
            for dd in range(2 * hb, 2 * hb + 2):
                nc.sync.dma_start(
                    out=q_sb[:, 8 * dd : 8 * (dd + 1), :],
                    in_=qsrc[:, 8 * dd : 8 * (dd + 1), :],
                )
            sl = slice(hb * H, (hb + 1) * H)
            for t in range(hb * H, (hb + 1) * H):
                nc.vector.bn_stats(out=qst[:, t, :], in_=q_sb[:, t, :])
            me, m2e = qst[:, sl, 1], qst[:, sl, 2]
            mo, m2o = qst[:, sl, 4], qst[:, sl, 5]
            nc.vector.tensor_tensor(out=mu[:, sl], in0=me, in1=mo, op=ALU.add)
            nc.vector.tensor_scalar_mul(out=mu[:, sl], in0=mu[:, sl], scalar1=0.5)
            nc.vector.tensor_tensor(out=dlt[:, sl], in0=me, in1=mo, op=ALU.subtract)
            nc.vector.tensor_tensor(out=dlt[:, sl], in0=dlt[:, sl], in1=dlt[:, sl], op=ALU.mult)
            nc.vector.tensor_tensor(out=v128[:, sl], in0=m2e, in1=m2o, op=ALU.add)
            nc.vector.scalar_tensor_tensor(
                out=v128[:, sl], in0=dlt[:, sl], scalar=32.0, in1=v128[:, sl],
                op0=ALU.mult, op1=ALU.add,
            )
            nc.scalar.activation(
                out=qlv[:, sl], in_=v128[:, sl], func=AF.Ln, bias=eps_col, scale=1.0 / C
            )
            nc.scalar.activation(
                out=qr[:, sl], in_=qlv[:, sl], func=AF.Exp, bias=0.0, scale=-0.5
            )
            nc.vector.scalar_tensor_tensor(
                out=negmur[:, sl], in0=mu[:, sl], scalar=-1.0, in1=qr[:, sl],
                op0=ALU.mult, op1=ALU.mult,
            )
            for t in range(hb * H, (hb + 1) * H):
                nc.gpsimd.tensor_scalar(
                    out=qln[:, t, :], in0=q_sb[:, t, :],
                    scalar1=mu[:, t : t + 1], scalar2=qr[:, t : t + 1],
                    op0=ALU.subtract, op1=ALU.mult,
                )

        def phase_b(j, hb):
            q_sb, qst, mu, qr, dlt, v128, qlv, negmur, qln, o_sb = state[j]
            for g2 in range(hb * (QG // 4), (hb + 1) * (QG // 4)):
                p_qt = ps_qt.tile([128, 8, 128], F16, tag="p_qt")
                for i in range(8):
                    nc.tensor.transpose(
                        p_qt[:, i, :], qln[:, 8 * g2 + i, :], ident16
                    )
                qlnT = mid.tile([128, 2, 512], F16, tag="qlnT")
                for half in range(2):
                    src = p_qt[:, 4 * half : 4 * half + 4, :].rearrange(
                        "p a b -> p (a b)"
                    )
                    nc.scalar.copy(out=qlnT[:, half, :], in_=src)
                p_st = ps_st.tile([128, 512], F32, tag="p_st")
                nc.tensor.matmul(
                    p_st[0:M, :], kx_all[:, j, :], qlnT[:, 0, :],
                    start=True, stop=True,
                )
                nc.tensor.matmul(
                    p_st[M:128, :], kx_all[:, j, :], qlnT[:, 1, :],
                    start=True, stop=True,
                    tile_position=(0, 64), skip_group_check=True,
                )
                et = mid.tile([128, 512], F16, tag="et")
                nc.scalar.activation(
                    out=et, in_=p_st, func=AF.Exp,
                    bias=tcol_all[:, j : j + 1], scale=1.0,
                )
                rden = small.tile([128, 8], F32, tag="rden")
                for k in range(2):
                    p_o = ps_ou.tile([128, 2, 512], F32, tag="p_o")
                    for half in range(2):
                        pb = M * half
                        for i in range(2):
                            t = 2 * k + i
                            nc.tensor.matmul(
                                p_o[:, half, 132 * i : 132 * i + 129],
                                et[pb : pb + M, 128 * t : 128 * (t + 1)],
                                v2_all[pb : pb + M, j, :],
                                start=True, stop=True,
                                tile_position=(pb, 0),
                            )
                    pov = p_o[:, :, 0:264].rearrange("p h (x c) -> p h x c", x=2)
                    nc.vector.reciprocal(
                        out=rden[:, 4 * k : 4 * k + 4],
                        in_=pov[:, :, :, 128],
                    )
                    osl = (
                        o_sb[:, 8 * g2 : 8 * g2 + 8, :]
                        .rearrange("p (h x) c -> p h x c", h=2)[:, :, 2 * k : 2 * k + 2, :]
                    )
                    if (j * (QG // 2) + g2) % 3 == 1:
                        for hh in range(2):
                            for xx in range(2):
                                nc.scalar.activation(
                                    out=osl[:, hh, xx, :],
                                    in_=pov[:, hh, xx, 0:128],
                                    func=AF.Copy, bias=0.0,
                                    scale=rden[:, 4 * k + 2 * hh + xx : 4 * k + 2 * hh + xx + 1],
                                )
                    else:
                        nc.vector.tensor_tensor(
                            out=osl,
                            in0=pov[:, :, :, 0:128],
                            in1=rden[:, 4 * k : 4 * k + 4]
                            .rearrange("p (h x) -> p h x", h=2)
                            .unsqueeze(3)
                            .broadcast_to([128, 2, 2, C]),
                            op=ALU.mult,
                        )
            odst = out[j].rearrange("(p t) c -> p t c", t=TQ)
            nc.sync.dma_start(
                out=odst[:, 16 * hb : 16 * (hb + 1), :],
                in_=o_sb[:, 16 * hb : 16 * (hb + 1), :],
            )
            if hb == 1:
                state.pop(j)

        def prep_all():
            with tc.tile_pool(name="prep_ps", bufs=1, space="PSUM") as prep_ps:
                # Wk' = g1 (.) Wk  (gamma1 fold)
                wkg = singles.tile([128, 2, C], F32, tag="wkg")
                for h in range(2):
                    nc.vector.tensor_scalar_mul(
                        out=wkg[:, h, :], in0=wk_sb[:, h, :], scalar1=g1c[:, h : h + 1]
                    )
                # tk[c] = sum_e b1[e] Wk[e,c]  (beta1 fold)
                ps_tk = prep_ps.tile([C, 1], F32, tag="ps_tk")
                for h in range(2):
                    nc.tensor.matmul(
                        ps_tk, wk_sb[:, h, :], b1c[:, h : h + 1],
                        start=(h == 0), stop=(h == 1),
                    )
                tk_col = singles.tile([C, 1], F32, tag="tk")
                nc.vector.tensor_copy(tk_col, ps_tk)

                # ---- key/value rows: LN(key), transposes ----
                kv_sb = singles.tile([128, 8, E], F32, tag="kv")   # t<4: key, t>=4: value
                nc.sync.dma_start(
                    out=kv_sb[:, 0:4, :],
                    in_=key[:, :, :].flatten_outer_dims().rearrange("(t p) e -> p t e", p=128),
                )
                nc.sync.dma_start(
                    out=kv_sb[:, 4:8, :],
                    in_=value[:, :, :].flatten_outer_dims().rearrange("(t p) e -> p t e", p=128),
                )
                kst = singles.tile([128, 4, 6], F32, tag="kst")
                for t in range(4):
                    nc.vector.bn_stats(out=kst[:, t, :], in_=kv_sb[:, t, :])
                kmv = singles.tile([128, 4, 2], F32, tag="kmv")
                for t in range(4):
                    nc.vector.bn_aggr(out=kmv[:, t, :], in_=kst[:, t, :])
                klnv = singles.tile([128, 4], F32, tag="klnv")
                nc.scalar.activation(
                    out=klnv, in_=kmv[:, :, 1], func=AF.Ln, bias=eps_col, scale=1.0
                )
                krs = singles.tile([128, 4], F32, tag="krs")
                nc.scalar.activation(out=krs, in_=klnv, func=AF.Exp, bias=0.0, scale=-0.5)
                kln = singles.tile([128, 4, E], F32, tag="kln")
                for t in range(4):
                    nc.vector.tensor_scalar(
                        out=kln[:, t, :], in0=kv_sb[:, t, :],
                        scalar1=kmv[:, t, 0:1], scalar2=krs[:, t : t + 1],
                        op0=ALU.subtract, op1=ALU.mult,
                    )
                # transposes: [128 rows, 128 e] -> [128 e, 128 rows]
                klnT = singles.tile([128, 8, 128], F32, tag="klnT")  # u = 2t+h
                valT = singles.tile([128, 8, 128], F32, tag="valT")
                for which in range(2):  # 0: key, 1: value
                    for grp in range(2):  # t pairs (0,1) then (2,3)
                        ps_tr = prep_ps.tile([128, 4, 128], F32, tag="ps_tr")
                        for i in range(2):
                            t = grp * 2 + i
                            for h in range(2):
                                src_ap = (
                                    kln[:, t, 128 * h : 128 * (h + 1)]
                                    if which == 0
                                    else kv_sb[:, 4 + t, 128 * h : 128 * (h + 1)]
                                )
                                nc.tensor.transpose(ps_tr[:, 2 * i + h, :], src_ap, ident)
                        dst = klnT if which == 0 else valT
                        nc.scalar.copy(out=dst[:, 4 * grp : 4 * grp + 4, :], in_=ps_tr)

                # ---- per-batch projections ----
                for j in range(BPC):
                    u0 = 2 * (j // 2)
                    msl = slice(64 * (j % 2), 64 * (j % 2) + 64)
                    ps_k = prep_ps.tile([C, M], F32, tag="ps_k")
                    for h in range(2):
                        nc.tensor.matmul(
                            ps_k, wkg[:, h, :], klnT[:, u0 + h, msl],
                            start=(h == 0), stop=(h == 1),
                        )
                    kp_sb = singles.tile([C, M], F32, tag=f"kp{j}")
                    nc.vector.tensor_scalar_add(out=kp_sb, in0=ps_k, scalar1=tk_col)
                    nc.vector.tensor_scalar_mul(out=kx_all[:, j, :], in0=kp_sb, scalar1=g2c)
                    ps_t = prep_ps.tile([128, 1], F32, tag="ps_t")
                    nc.tensor.matmul(ps_t[0:M, :], kp_sb, b2c, start=True, stop=True)
                    nc.tensor.matmul(
                        ps_t[M : 2 * M, :], kp_sb, b2c, start=True, stop=True,
                        tile_position=(0, 64), skip_group_check=True,
                    )
                    nc.scalar.activation(
                        out=tcol_all[:, j : j + 1], in_=ps_t, func=AF.Copy,
                        bias=EXP_SHIFT, scale=1.0,
                    )
                    ps_v = prep_ps.tile([128, C], F32, tag="ps_v")
                    for h in range(2):
                        nc.tensor.matmul(
                            ps_v[0:M, :], valT[:, u0 + h, msl], wv_sb[:, h, :],
                            start=(h == 0), stop=(h == 1),
                        )
                    for h in range(2):
                        nc.tensor.matmul(
                            ps_v[M : 2 * M, :], valT[:, u0 + h, msl], wv_sb[:, h, :],
                            start=(h == 0), stop=(h == 1),
                            tile_position=(0, 64), skip_group_check=True,
                        )
                    nc.vector.tensor_copy(v2_all[:, j, 0:C], ps_v)
                nc.vector.memset(v2_all[:, :, C : C + 1], 1.0)

        steps = 2 * BPC
        SKEW = 2
        prep_all()
        ps_qt = ctx.enter_context(tc.tile_pool(name="ps_qt", bufs=2, space="PSUM"))
        ps_st = ctx.enter_context(tc.tile_pool(name="ps_st", bufs=2, space="PSUM"))
        ps_ou = ctx.enter_context(tc.tile_pool(name="ps_ou", bufs=2, space="PSUM"))
        for st in range(0, steps + SKEW):
            if st < steps:
                phase_a(st // 2, st % 2)
            if st >= SKEW:
                pj = st - SKEW
                phase_b(pj // 2, pj % 2)

    if split:
        _split_waits(nc, limit=1)
    return nc


_NC = None


def kernel(**inputs):
    global _NC
    if _NC is None:
        _NC = _build_nc()
    q = np.ascontiguousarray(np.asarray(inputs["query"], dtype=np.float32))
    k = np.ascontiguousarray(np.asarray(inputs["key"], dtype=np.float32))
    v = np.ascontiguousarray(np.asarray(inputs["value"], dtype=np.float32))
    shared = {
        "wk": np.ascontiguousarray(np.asarray(inputs["k_proj_weight"], np.float32)),
        "wv": np.ascontiguousarray(np.asarray(inputs["v_proj_weight"], np.float32)),
        "g1": np.ascontiguousarray(np.asarray(inputs["norm1_gamma"], np.float32)),
        "b1": np.ascontiguousarray(np.asarray(inputs["norm1_beta"], np.float32)),
        "g2": np.ascontiguousarray(np.asarray(inputs["norm2_gamma"], np.float32)),
        "b2": np.ascontiguousarray(np.asarray(inputs["norm2_beta"], np.float32)),
    }
    in_maps = []
    for c in range(NCORES):
        sl = slice(c * BPC, (c + 1) * BPC)
        in_maps.append({"query": q[sl], "key": k[sl], "value": v[sl], **shared})
    res = run_bass_kernel_spmd(_NC, in_maps, core_ids=list(range(NCORES)))
    return np.concatenate([res.results[i]["out"] for i in range(NCORES)], axis=0)



# revision 51
# speedup vs baseline: 1.0192x; 1.0192x over previous
"""AttnDecoderBlock on 8 TRN2 NeuronCores — data-parallel over batch.

Per batch b:
  k   = LN_E(key[b]) ; kp = einsum('me,ec->cm', k, Wk)
  q   = LN_C(query[b])
  att = softmax(q @ kp, axis=-1)
  out = att @ (value[b] @ Wv)

Sharding: batch 64 -> 8 cores x 8 batches each. Weights/params replicated.

Per-core pipeline (row-major orientation, query rows on partitions):
  - query row r of a batch lives at partition r//32, tile r%32, so every DMA
    moves 16KB contiguous per partition (max DMA efficiency).
  - LN stats: per-tile bn_stats + batched even/odd Chan combine.
  - rsqrt(var) as Exp(-0.5*Ln(var+eps)) so the only ACT table set used is
    natural_log_exp_and_others (Ln/Exp/Copy/Identity) — no table thrash.
  - LN apply on GPSIMD (tensor_scalar), freeing DVE/ACT.
  - PE transposes q_ln tiles; ACT copies PSUM->SBUF casting to fp32r.
  - scores^T = Kx^T @ qlnT in fp32r (full rate at 512 moving columns).
  - softmax exp on ACT, bias = beta2-fold t_m - 15 (shift cancels in the
    normalize), output fp16 (fits after shift; 8x finer mantissa than bf16).
  - out tile = E_t @ [Vproj | 1] in fp16; denominator lands as column 128.
  - normalize on DVE: reciprocal + broadcast tensor_tensor from PSUM.
LN gamma/beta folds: g1 into Wk, b1 into tk column, g2 into Kx, b2 into the
exp bias t_m. value path has no LN.
"""

import numpy as np

import concourse.bass as bass
import concourse.mybir as mybir
import concourse.tile as tile
from concourse.bass_utils import run_bass_kernel_spmd
from concourse.masks import make_identity

B, N, M, E, C = 64, 4096, 64, 256, 128
NCORES = 8
BPC = B // NCORES          # batches per core
TQ = N // 128              # 32 row-tiles per batch
QG = TQ // 4               # 8 quads per batch
EPS = 1e-5
EXP_SHIFT = -15.0          # softmax shift; cancels in normalization
F32 = mybir.dt.float32
F16 = mybir.dt.float16

_ctr = [0]


def _split_waits(nc, limit=1):
    """The axon-path walrus accepts only `limit` sem-waits per instruction;
    move excess onto preceding same-engine NOPs (program order on the engine
    makes this equivalent)."""
    for f in nc.m.functions:
        for bb in f.blocks:
            out = []
            changed = False
            for inst in bb.instructions:
                si = inst.sync_info
                if si is not None and si.on_wait is not None and len(si.on_wait) > limit:
                    waits = list(si.on_wait)
                    while len(waits) > limit:
                        chunk, waits = waits[:limit], waits[limit:]
                        _ctr[0] += 1
                        nop = mybir.InstNoOp(name=f"I-wsplit-{_ctr[0]}", ins=[], outs=[])
                        nop.engine = inst.engine
                        nop.sync_info = mybir.SyncInfo(on_wait=chunk, on_update=[])
                        out.append(nop)
                        changed = True
                    inst.sync_info = mybir.SyncInfo(on_wait=waits, on_update=si.on_update)
                out.append(inst)
            if changed:
                bb.instructions = out
    return nc


def _build_nc(split=True):
    nc = bass.Bass()
    AF = mybir.ActivationFunctionType
    ALU = mybir.AluOpType

    query = nc.dram_tensor("query", [BPC, N, C], F32, kind="ExternalInput")
    key = nc.dram_tensor("key", [BPC, M, E], F32, kind="ExternalInput")
    value = nc.dram_tensor("value", [BPC, M, E], F32, kind="ExternalInput")
    wk = nc.dram_tensor("wk", [E, C], F32, kind="ExternalInput")
    wv = nc.dram_tensor("wv", [E, C], F32, kind="ExternalInput")
    g1 = nc.dram_tensor("g1", [E], F32, kind="ExternalInput")
    b1 = nc.dram_tensor("b1", [E], F32, kind="ExternalInput")
    g2 = nc.dram_tensor("g2", [C], F32, kind="ExternalInput")
    b2 = nc.dram_tensor("b2", [C], F32, kind="ExternalInput")
    out = nc.dram_tensor("out", [BPC, N, C], F32, kind="ExternalOutput")

    import contextlib
    with tile.TileContext(nc) as tc, contextlib.ExitStack() as ctx:
        singles = ctx.enter_context(tc.tile_pool(name="singles", bufs=1))

        ident = singles.tile([128, 128], F32, tag="ident")
        make_identity(nc, ident)
        ident16 = singles.tile([128, 128], F16, tag="ident16")
        nc.gpsimd.tensor_copy(ident16, ident)
        eps_col = singles.tile([128, 1], F32, tag="eps")
        nc.vector.memset(eps_col, EPS)

        # ---- weights / params ----
        wk_sb = singles.tile([128, 2, C], F32, tag="wk")       # [p, h, c], e=128h+p
        wv_sb = singles.tile([128, 2, C], F32, tag="wv")
        nc.sync.dma_start(out=wk_sb, in_=wk.rearrange("(h p) c -> p h c", p=128))
        nc.sync.dma_start(out=wv_sb, in_=wv.rearrange("(h p) c -> p h c", p=128))
        g1c = singles.tile([128, 2], F32, tag="g1")
        b1c = singles.tile([128, 2], F32, tag="b1")
        nc.sync.dma_start(out=g1c, in_=g1.rearrange("(h p) -> p h", p=128))
        nc.sync.dma_start(out=b1c, in_=b1.rearrange("(h p) -> p h", p=128))
        g2c = singles.tile([128, 1], F32, tag="g2")
        b2c = singles.tile([128, 1], F32, tag="b2")
        nc.sync.dma_start(out=g2c, in_=g2[:].unsqueeze(1))
        nc.sync.dma_start(out=b2c, in_=b2[:].unsqueeze(1))

        kx_all = singles.tile([128, BPC, M], F16, tag="kx")     # g2 (.) KP
        tcol_all = singles.tile([128, BPC], F32, tag="tcol")    # t_m + shift, dup rows
        v2_all = singles.tile([128, BPC, C + 1], F16, tag="v2") # [Vproj | 1], dup rows

        # ---- main loop ----
        qpool = ctx.enter_context(tc.tile_pool(name="qpool", bufs=3))
        opool = ctx.enter_context(tc.tile_pool(name="opool", bufs=3))
        small = ctx.enter_context(tc.tile_pool(name="small", bufs=3))
        mid = ctx.enter_context(tc.tile_pool(name="mid", bufs=6))

        state = {}
        H = TQ // 2

        def phase_a(j, hb):
            if hb == 0:
                q_sb = qpool.tile([128, TQ, C], F32, tag="q")
                qst = small.tile([128, TQ, 6], F32, tag="qst")
                mu = small.tile([128, TQ], F32, tag="mu")
                qr = small.tile([128, TQ], F32, tag="qr")
                dlt = small.tile([128, TQ], F32, tag="dlt")
                v128 = small.tile([128, TQ], F32, tag="v128")
                qlv = small.tile([128, TQ], F32, tag="qlv")
                qln = qpool.tile([128, TQ, C], F16, tag="qln")
                o_sb = opool.tile([128, TQ, C], F32, tag="o")
                state[j] = (q_sb, qst, mu, qr, dlt, v128, qlv, qln, o_sb)
            q_sb, qst, mu, qr, dlt, v128, qlv, qln, o_sb = state[j]
            qsrc = query[j].rearrange("(p t) c -> p t c", t=TQ)
            for dd in range(2 * hb, 2 * hb + 2):
                nc.sync.dma_start(
                    out=q_sb[:, 8 * dd : 8 * (dd + 1), :],
                    in_=qsrc[:, 8 * dd : 8 * (dd + 1), :],
                )
            sl = slice(hb * H, (hb + 1) * H)
            for t in range(hb * H, (hb + 1) * H):
                nc.vector.bn_stats(out=qst[:, t, :], in_=q_sb[:, t, :])
            me, m2e = qst[:, sl, 1], qst[:, sl, 2]
            mo, m2o = qst[:, sl, 4], qst[:, sl, 5]
            nc.vector.tensor_tensor(out=mu[:, sl], in0=me, in1=mo, op=ALU.add)
            nc.vector.tensor_scalar_mul(out=mu[:, sl], in0=mu[:, sl], scalar1=0.5)
            nc.vector.tensor_tensor(out=dlt[:, sl], in0=me, in1=mo, op=ALU.subtract)
            nc.vector.tensor_tensor(out=dlt[:, sl], in0=dlt[:, sl], in1=dlt[:, sl], op=ALU.mult)
            nc.vector.tensor_tensor(out=v128[:, sl], in0=m2e, in1=m2o, op=ALU.add)
            nc.vector.scalar_tensor_tensor(
                out=v128[:, sl], in0=dlt[:, sl], scalar=32.0, in1=v128[:, sl],
                op0=ALU.mult, op1=ALU.add,
            )
            nc.scalar.activation(
                out=qlv[:, sl], in_=v128[:, sl], func=AF.Ln, bias=eps_col, scale=1.0 / C
            )
            nc.scalar.activation(
                out=qr[:, sl], in_=qlv[:, sl], func=AF.Exp, bias=0.0, scale=-0.5
            )
            for t in range(hb * H, (hb + 1) * H):
                nc.gpsimd.tensor_scalar(
                    out=qln[:, t, :], in0=q_sb[:, t, :],
                    scalar1=mu[:, t : t + 1], scalar2=qr[:, t : t + 1],
                    op0=ALU.subtract, op1=ALU.mult,
                )

        def phase_b(j, hb):
            q_sb, qst, mu, qr, dlt, v128, qlv, qln, o_sb = state[j]
            for g2 in range(hb * (QG // 4), (hb + 1) * (QG // 4)):
                p_qt = ps_qt.tile([128, 8, 128], F16, tag="p_qt")
                for i in range(8):
                    nc.tensor.transpose(
                        p_qt[:, i, :], qln[:, 8 * g2 + i, :], ident16
                    )
                qlnT = mid.tile([128, 2, 512], F16, tag="qlnT")
                for half in range(2):
                    src = p_qt[:, 4 * half : 4 * half + 4, :].rearrange(
                        "p a b -> p (a b)"
                    )
                    nc.scalar.copy(out=qlnT[:, half, :], in_=src)
                p_st = ps_st.tile([128, 512], F32, tag="p_st")
                nc.tensor.matmul(
                    p_st[0:M, :], kx_all[:, j, :], qlnT[:, 0, :],
                    start=True, stop=True,
                )
                nc.tensor.matmul(
                    p_st[M:128, :], kx_all[:, j, :], qlnT[:, 1, :],
                    start=True, stop=True,
                    tile_position=(0, 64), skip_group_check=True,
                )
                et = mid.tile([128, 512], F16, tag="et")
                nc.scalar.activation(
                    out=et, in_=p_st, func=AF.Exp,
                    bias=tcol_all[:, j : j + 1], scale=1.0,
                )
                rden = small.tile([128, 8], F32, tag="rden")
                for k in range(2):
                    p_o = ps_ou.tile([128, 2, 512], F32, tag="p_o")
                    for half in range(2):
                        pb = M * half
                        for i in range(2):
                            t = 2 * k + i
                            nc.tensor.matmul(
                                p_o[:, half, 132 * i : 132 * i + 129],
                                et[pb : pb + M, 128 * t : 128 * (t + 1)],
                                v2_all[pb : pb + M, j, :],
                                start=True, stop=True,
                                tile_position=(pb, 0),
                            )
                    pov = p_o[:, :, 0:264].rearrange("p h (x c) -> p h x c", x=2)
                    nc.vector.reciprocal(
                        out=rden[:, 4 * k : 4 * k + 4],
                        in_=pov[:, :, :, 128],
                    )
                    osl = (
                        o_sb[:, 8 * g2 : 8 * g2 + 8, :]
                        .rearrange("p (h x) c -> p h x c", h=2)[:, :, 2 * k : 2 * k + 2, :]
                    )
                    if (j * (QG // 2) + g2) % 2 == 1:
                        for hh in range(2):
                            for xx in range(2):
                                nc.scalar.activation(
                                    out=osl[:, hh, xx, :],
                                    in_=pov[:, hh, xx, 0:128],
                                    func=AF.Copy, bias=0.0,
                                    scale=rden[:, 4 * k + 2 * hh + xx : 4 * k + 2 * hh + xx + 1],
                                )
                    else:
                        nc.vector.tensor_tensor(
                            out=osl,
                            in0=pov[:, :, :, 0:128],
                            in1=rden[:, 4 * k : 4 * k + 4]
                            .rearrange("p (h x) -> p h x", h=2)
                            .unsqueeze(3)
                            .broadcast_to([128, 2, 2, C]),
                            op=ALU.mult,
                        )
            odst = out[j].rearrange("(p t) c -> p t c", t=TQ)
            for qq in range(4 * hb, 4 * hb + 4):
                nc.sync.dma_start(
                    out=odst[:, 4 * qq : 4 * (qq + 1), :],
                    in_=o_sb[:, 4 * qq : 4 * (qq + 1), :],
                )
            if hb == 1:
                state.pop(j)

        def prep_all():
            with tc.tile_pool(name="prep_ps", bufs=1, space="PSUM") as prep_ps:
                # Wk' = g1 (.) Wk  (gamma1 fold)
                wkg = singles.tile([128, 2, C], F32, tag="wkg")
                for h in range(2):
                    nc.vector.tensor_scalar_mul(
                        out=wkg[:, h, :], in0=wk_sb[:, h, :], scalar1=g1c[:, h : h + 1]
                    )
                # tk[c] = sum_e b1[e] Wk[e,c]  (beta1 fold)
                ps_tk = prep_ps.tile([C, 1], F32, tag="ps_tk")
                for h in range(2):
                    nc.tensor.matmul(
                        ps_tk, wk_sb[:, h, :], b1c[:, h : h + 1],
                        start=(h == 0), stop=(h == 1),
                    )
                tk_col = singles.tile([C, 1], F32, tag="tk")
                nc.vector.tensor_copy(tk_col, ps_tk)

                # ---- key/value rows: LN(key), transposes ----
                kv_sb = singles.tile([128, 8, E], F32, tag="kv")   # t<4: key, t>=4: value
                nc.sync.dma_start(
                    out=kv_sb[:, 0:4, :],
                    in_=key[:, :, :].flatten_outer_dims().rearrange("(t p) e -> p t e", p=128),
                )
                nc.sync.dma_start(
                    out=kv_sb[:, 4:8, :],
                    in_=value[:, :, :].flatten_outer_dims().rearrange("(t p) e -> p t e", p=128),
                )
                kst = singles.tile([128, 4, 6], F32, tag="kst")
                for t in range(4):
                    nc.vector.bn_stats(out=kst[:, t, :], in_=kv_sb[:, t, :])
                kmv = singles.tile([128, 4, 2], F32, tag="kmv")
                for t in range(4):
                    nc.vector.bn_aggr(out=kmv[:, t, :], in_=kst[:, t, :])
                klnv = singles.tile([128, 4], F32, tag="klnv")
                nc.scalar.activation(
                    out=klnv, in_=kmv[:, :, 1], func=AF.Ln, bias=eps_col, scale=1.0
                )
                krs = singles.tile([128, 4], F32, tag="krs")
                nc.scalar.activation(out=krs, in_=klnv, func=AF.Exp, bias=0.0, scale=-0.5)
                kln = singles.tile([128, 4, E], F32, tag="kln")
                for t in range(4):
                    nc.vector.tensor_scalar(
                        out=kln[:, t, :], in0=kv_sb[:, t, :],
                        scalar1=kmv[:, t, 0:1], scalar2=krs[:, t : t + 1],
                        op0=ALU.subtract, op1=ALU.mult,
                    )
                # transposes: [128 rows, 128 e] -> [128 e, 128 rows]
                klnT = singles.tile([128, 8, 128], F32, tag="klnT")  # u = 2t+h
                valT = singles.tile([128, 8, 128], F32, tag="valT")
                for which in range(2):  # 0: key, 1: value
                    for grp in range(2):  # t pairs (0,1) then (2,3)
                        ps_tr = prep_ps.tile([128, 4, 128], F32, tag="ps_tr")
                        for i in range(2):
                            t = grp * 2 + i
                            for h in range(2):
                                src_ap = (
                                    kln[:, t, 128 * h : 128 * (h + 1)]
                                    if which == 0
                                    else kv_sb[:, 4 + t, 128 * h : 128 * (h + 1)]
                                )
                                nc.tensor.transpose(ps_tr[:, 2 * i + h, :], src_ap, ident)
                        dst = klnT if which == 0 else valT
                        nc.scalar.copy(out=dst[:, 4 * grp : 4 * grp + 4, :], in_=ps_tr)

                # ---- per-batch projections ----
                for j in range(BPC):
                    u0 = 2 * (j // 2)
                    msl = slice(64 * (j % 2), 64 * (j % 2) + 64)
                    ps_k = prep_ps.tile([C, M], F32, tag="ps_k")
                    for h in range(2):
                        nc.tensor.matmul(
                            ps_k, wkg[:, h, :], klnT[:, u0 + h, msl],
                            start=(h == 0), stop=(h == 1),
                        )
                    kp_sb = singles.tile([C, M], F32, tag=f"kp{j}")
                    nc.vector.tensor_scalar_add(out=kp_sb, in0=ps_k, scalar1=tk_col)
                    nc.vector.tensor_scalar_mul(out=kx_all[:, j, :], in0=kp_sb, scalar1=g2c)
                    ps_t = prep_ps.tile([128, 1], F32, tag="ps_t")
                    nc.tensor.matmul(ps_t[0:M, :], kp_sb, b2c, start=True, stop=True)
                    nc.tensor.matmul(
                        ps_t[M : 2 * M, :], kp_sb, b2c, start=True, stop=True,
                        tile_position=(0, 64), skip_group_check=True,
                    )
                    nc.scalar.activation(
                        out=tcol_all[:, j : j + 1], in_=ps_t, func=AF.Copy,
                        bias=EXP_SHIFT, scale=1.0,
                    )
                    ps_v = prep_ps.tile([128, C], F32, tag="ps_v")
                    for h in range(2):
                        nc.tensor.matmul(
                            ps_v[0:M, :], valT[:, u0 + h, msl], wv_sb[:, h, :],
                            start=(h == 0), stop=(h == 1),
                        )
                    for h in range(2):
                        nc.tensor.matmul(
                            ps_v[M : 2 * M, :], valT[:, u0 + h, msl], wv_sb[:, h, :],
                            start=(h == 0), stop=(h == 1),
                            tile_position=(0, 64), skip_group_check=True,
                        )
                    nc.vector.tensor_copy(v2_all[:, j, 0:C], ps_v)
                nc.vector.memset(v2_all[:, :, C : C + 1], 1.0)

        steps = 2 * BPC
        SKEW = 3
        prep_all()
        ps_qt = ctx.enter_context(tc.tile_pool(name="ps_qt", bufs=2, space="PSUM"))
        ps_st = ctx.enter_context(tc.tile_pool(name="ps_st", bufs=2, space="PSUM"))
        ps_ou = ctx.enter_context(tc.tile_pool(name="ps_ou", bufs=2, space="PSUM"))
        for st in range(0, steps + SKEW):
            if st < steps:
                phase_a(st // 2, st % 2)
            if st >= SKEW:
                pj = st - SKEW
                phase_b(pj // 2, pj % 2)

    if split:
        _split_waits(nc, limit=1)
    return nc


_NC = None


def kernel(**inputs):
    global _NC
    if _NC is None:
        _NC = _build_nc()
    q = np.ascontiguousarray(np.asarray(inputs["query"], dtype=np.float32))
    k = np.ascontiguousarray(np.asarray(inputs["key"], dtype=np.float32))
    v = np.ascontiguousarray(np.asarray(inputs["value"], dtype=np.float32))
    shared = {
        "wk": np.ascontiguousarray(np.asarray(inputs["k_proj_weight"], np.float32)),
        "wv": np.ascontiguousarray(np.asarray(inputs["v_proj_weight"], np.float32)),
        "g1": np.ascontiguousarray(np.asarray(inputs["norm1_gamma"], np.float32)),
        "b1": np.ascontiguousarray(np.asarray(inputs["norm1_beta"], np.float32)),
        "g2": np.ascontiguousarray(np.asarray(inputs["norm2_gamma"], np.float32)),
        "b2": np.ascontiguousarray(np.asarray(inputs["norm2_beta"], np.float32)),
    }
    in_maps = []
    for c in range(NCORES):
        sl = slice(c * BPC, (c + 1) * BPC)
        in_maps.append({"query": q[sl], "key": k[sl], "value": v[sl], **shared})
    res = run_bass_kernel_spmd(_NC, in_maps, core_ids=list(range(NCORES)))
    return np.concatenate([res.results[i]["out"] for i in range(NCORES)], axis=0)



# revision 57
# speedup vs baseline: 1.0246x; 1.0054x over previous
"""AttnDecoderBlock on 8 TRN2 NeuronCores — data-parallel over batch.

Per batch b:
  k   = LN_E(key[b]) ; kp = einsum('me,ec->cm', k, Wk)
  q   = LN_C(query[b])
  att = softmax(q @ kp, axis=-1)
  out = att @ (value[b] @ Wv)

Sharding: batch 64 -> 8 cores x 8 batches each. Weights/params replicated.

Per-core pipeline (row-major orientation, query rows on partitions):
  - query row r of a batch lives at partition r//32, tile r%32, so every DMA
    moves 16KB contiguous per partition (max DMA efficiency).
  - LN stats: per-tile bn_stats + batched even/odd Chan combine.
  - rsqrt(var) as Exp(-0.5*Ln(var+eps)) so the only ACT table set used is
    natural_log_exp_and_others (Ln/Exp/Copy/Identity) — no table thrash.
  - LN apply on GPSIMD (tensor_scalar), freeing DVE/ACT.
  - PE transposes q_ln tiles; ACT copies PSUM->SBUF casting to fp32r.
  - scores^T = Kx^T @ qlnT in fp32r (full rate at 512 moving columns).
  - softmax exp on ACT, bias = beta2-fold t_m - 15 (shift cancels in the
    normalize), output fp16 (fits after shift; 8x finer mantissa than bf16).
  - out tile = E_t @ [Vproj | 1] in fp16; denominator lands as column 128.
  - normalize on DVE: reciprocal + broadcast tensor_tensor from PSUM.
LN gamma/beta folds: g1 into Wk, b1 into tk column, g2 into Kx, b2 into the
exp bias t_m. value path has no LN.
"""

import numpy as np

import concourse.bass as bass
import concourse.mybir as mybir
import concourse.tile as tile
from concourse.bass_utils import run_bass_kernel_spmd
from concourse.masks import make_identity

B, N, M, E, C = 64, 4096, 64, 256, 128
NCORES = 8
BPC = B // NCORES          # batches per core
TQ = N // 128              # 32 row-tiles per batch
QG = TQ // 4               # 8 quads per batch
EPS = 1e-5
EXP_SHIFT = -15.0          # softmax shift; cancels in normalization
F32 = mybir.dt.float32
F16 = mybir.dt.float16

_ctr = [0]


def _split_waits(nc, limit=1):
    """The axon-path walrus accepts only `limit` sem-waits per instruction;
    move excess onto preceding same-engine NOPs (program order on the engine
    makes this equivalent)."""
    for f in nc.m.functions:
        for bb in f.blocks:
            out = []
            changed = False
            for inst in bb.instructions:
                si = inst.sync_info
                if si is not None and si.on_wait is not None and len(si.on_wait) > limit:
                    waits = list(si.on_wait)
                    while len(waits) > limit:
                        chunk, waits = waits[:limit], waits[limit:]
                        _ctr[0] += 1
                        nop = mybir.InstNoOp(name=f"I-wsplit-{_ctr[0]}", ins=[], outs=[])
                        nop.engine = inst.engine
                        nop.sync_info = mybir.SyncInfo(on_wait=chunk, on_update=[])
                        out.append(nop)
                        changed = True
                    inst.sync_info = mybir.SyncInfo(on_wait=waits, on_update=si.on_update)
                out.append(inst)
            if changed:
                bb.instructions = out
    return nc


def _build_nc(split=True):
    nc = bass.Bass()
    AF = mybir.ActivationFunctionType
    ALU = mybir.AluOpType

    query = nc.dram_tensor("query", [BPC, N, C], F32, kind="ExternalInput")
    key = nc.dram_tensor("key", [BPC, M, E], F32, kind="ExternalInput")
    value = nc.dram_tensor("value", [BPC, M, E], F32, kind="ExternalInput")
    wk = nc.dram_tensor("wk", [E, C], F32, kind="ExternalInput")
    wv = nc.dram_tensor("wv", [E, C], F32, kind="ExternalInput")
    g1 = nc.dram_tensor("g1", [E], F32, kind="ExternalInput")
    b1 = nc.dram_tensor("b1", [E], F32, kind="ExternalInput")
    g2 = nc.dram_tensor("g2", [C], F32, kind="ExternalInput")
    b2 = nc.dram_tensor("b2", [C], F32, kind="ExternalInput")
    out = nc.dram_tensor("out", [BPC, N, C], F32, kind="ExternalOutput")

    import contextlib
    with tile.TileContext(nc) as tc, contextlib.ExitStack() as ctx:
        singles = ctx.enter_context(tc.tile_pool(name="singles", bufs=1))

        ident = singles.tile([128, 128], F32, tag="ident")
        make_identity(nc, ident)
        ident16 = singles.tile([128, 128], F16, tag="ident16")
        nc.gpsimd.tensor_copy(ident16, ident)
        eps_col = singles.tile([128, 1], F32, tag="eps")
        nc.vector.memset(eps_col, EPS)

        # ---- weights / params ----
        wk_sb = singles.tile([128, 2, C], F32, tag="wk")       # [p, h, c], e=128h+p
        wv_sb = singles.tile([128, 2, C], F32, tag="wv")
        nc.sync.dma_start(out=wk_sb, in_=wk.rearrange("(h p) c -> p h c", p=128))
        nc.sync.dma_start(out=wv_sb, in_=wv.rearrange("(h p) c -> p h c", p=128))
        g1c = singles.tile([128, 2], F32, tag="g1")
        b1c = singles.tile([128, 2], F32, tag="b1")
        nc.sync.dma_start(out=g1c, in_=g1.rearrange("(h p) -> p h", p=128))
        nc.sync.dma_start(out=b1c, in_=b1.rearrange("(h p) -> p h", p=128))
        g2c = singles.tile([128, 1], F32, tag="g2")
        b2c = singles.tile([128, 1], F32, tag="b2")
        nc.sync.dma_start(out=g2c, in_=g2[:].unsqueeze(1))
        nc.sync.dma_start(out=b2c, in_=b2[:].unsqueeze(1))

        kx_all = singles.tile([128, BPC, M], F16, tag="kx")     # g2 (.) KP
        tcol_all = singles.tile([128, BPC], F32, tag="tcol")    # t_m + shift, dup rows
        v2_all = singles.tile([128, BPC, C + 1], F16, tag="v2") # [Vproj | 1], dup rows

        # ---- main loop ----
        qpool = ctx.enter_context(tc.tile_pool(name="qpool", bufs=3))
        opool = ctx.enter_context(tc.tile_pool(name="opool", bufs=3))
        small = ctx.enter_context(tc.tile_pool(name="small", bufs=3))
        mid = ctx.enter_context(tc.tile_pool(name="mid", bufs=6))

        state = {}
        H = TQ // 2

        def phase_a(j, hb):
            if hb == 0:
                q_sb = qpool.tile([128, TQ, C], F32, tag="q")
                qst = small.tile([128, TQ, 6], F32, tag="qst")
                mu = small.tile([128, TQ], F32, tag="mu")
                qr = small.tile([128, TQ], F32, tag="qr")
                dlt = small.tile([128, TQ], F32, tag="dlt")
                v128 = small.tile([128, TQ], F32, tag="v128")
                qlv = small.tile([128, TQ], F32, tag="qlv")
                qln = qpool.tile([128, TQ, C], F16, tag="qln")
                o_sb = opool.tile([128, TQ, C], F32, tag="o")
                state[j] = (q_sb, qst, mu, qr, dlt, v128, qlv, qln, o_sb)
            q_sb, qst, mu, qr, dlt, v128, qlv, qln, o_sb = state[j]
            qsrc = query[j].rearrange("(p t) c -> p t c", t=TQ)
            for dd in range(2 * hb, 2 * hb + 2):
                nc.sync.dma_start(
                    out=q_sb[:, 8 * dd : 8 * (dd + 1), :],
                    in_=qsrc[:, 8 * dd : 8 * (dd + 1), :],
                )
            sl = slice(hb * H, (hb + 1) * H)
            for t in range(hb * H, (hb + 1) * H):
                nc.vector.bn_stats(out=qst[:, t, :], in_=q_sb[:, t, :])
            me, m2e = qst[:, sl, 1], qst[:, sl, 2]
            mo, m2o = qst[:, sl, 4], qst[:, sl, 5]
            nc.vector.tensor_tensor(out=mu[:, sl], in0=me, in1=mo, op=ALU.add)
            nc.vector.tensor_scalar_mul(out=mu[:, sl], in0=mu[:, sl], scalar1=0.5)
            nc.vector.tensor_tensor(out=dlt[:, sl], in0=me, in1=mo, op=ALU.subtract)
            nc.vector.tensor_tensor(out=dlt[:, sl], in0=dlt[:, sl], in1=dlt[:, sl], op=ALU.mult)
            nc.vector.tensor_tensor(out=v128[:, sl], in0=m2e, in1=m2o, op=ALU.add)
            nc.gpsimd.scalar_tensor_tensor(
                out=v128[:, sl], in0=dlt[:, sl], scalar=32.0, in1=v128[:, sl],
                op0=ALU.mult, op1=ALU.add,
            )
            nc.scalar.activation(
                out=qlv[:, sl], in_=v128[:, sl], func=AF.Ln, bias=eps_col, scale=1.0 / C
            )
            nc.scalar.activation(
                out=qr[:, sl], in_=qlv[:, sl], func=AF.Exp, bias=0.0, scale=-0.5
            )
            for t in range(hb * H, (hb + 1) * H):
                nc.gpsimd.tensor_scalar(
                    out=qln[:, t, :], in0=q_sb[:, t, :],
                    scalar1=mu[:, t : t + 1], scalar2=qr[:, t : t + 1],
                    op0=ALU.subtract, op1=ALU.mult,
                )

        def phase_b(j, hb):
            q_sb, qst, mu, qr, dlt, v128, qlv, qln, o_sb = state[j]
            for g2 in range(hb * (QG // 4), (hb + 1) * (QG // 4)):
                p_qt = ps_qt.tile([128, 8, 128], F16, tag="p_qt")
                for i in range(8):
                    nc.tensor.transpose(
                        p_qt[:, i, :], qln[:, 8 * g2 + i, :], ident16
                    )
                qlnT = mid.tile([128, 2, 512], F16, tag="qlnT")
                for half in range(2):
                    src = p_qt[:, 4 * half : 4 * half + 4, :].rearrange(
                        "p a b -> p (a b)"
                    )
                    nc.scalar.copy(out=qlnT[:, half, :], in_=src)
                p_st = ps_st.tile([128, 512], F32, tag="p_st")
                nc.tensor.matmul(
                    p_st[0:M, :], kx_all[:, j, :], qlnT[:, 0, :],
                    start=True, stop=True,
                )
                nc.tensor.matmul(
                    p_st[M:128, :], kx_all[:, j, :], qlnT[:, 1, :],
                    start=True, stop=True,
                    tile_position=(0, 64), skip_group_check=True,
                )
                et = mid.tile([128, 512], F16, tag="et")
                nc.scalar.activation(
                    out=et, in_=p_st, func=AF.Exp,
                    bias=tcol_all[:, j : j + 1], scale=1.0,
                )
                rden = small.tile([128, 8], F32, tag="rden")
                for k in range(2):
                    p_o = ps_ou.tile([128, 2, 512], F32, tag="p_o")
                    for half in range(2):
                        pb = M * half
                        for i in range(2):
                            t = 2 * k + i
                            nc.tensor.matmul(
                                p_o[:, half, 132 * i + 128 : 132 * i + 129],
                                et[pb : pb + M, 128 * t : 128 * (t + 1)],
                                v2_all[pb : pb + M, j, C : C + 1],
                                start=True, stop=True,
                                tile_position=(pb, 0), skip_group_check=True,
                            )
                    pov = p_o[:, :, 0:264].rearrange("p h (x c) -> p h x c", x=2)
                    nc.vector.reciprocal(
                        out=rden[:, 4 * k : 4 * k + 4],
                        in_=pov[:, :, :, 128],
                    )
                    for half in range(2):
                        pb = M * half
                        for i in range(2):
                            t = 2 * k + i
                            nc.tensor.matmul(
                                p_o[:, half, 132 * i : 132 * i + 128],
                                et[pb : pb + M, 128 * t : 128 * (t + 1)],
                                v2_all[pb : pb + M, j, 0:C],
                                start=True, stop=True,
                                tile_position=(pb, 0), skip_group_check=True,
                            )
                    osl = (
                        o_sb[:, 8 * g2 : 8 * g2 + 8, :]
                        .rearrange("p (h x) c -> p h x c", h=2)[:, :, 2 * k : 2 * k + 2, :]
                    )
                    if (j * (QG // 2) + g2) % 2 == 1:
                        for hh in range(2):
                            for xx in range(2):
                                nc.scalar.activation(
                                    out=osl[:, hh, xx, :],
                                    in_=pov[:, hh, xx, 0:128],
                                    func=AF.Copy, bias=0.0,
                                    scale=rden[:, 4 * k + 2 * hh + xx : 4 * k + 2 * hh + xx + 1],
                                )
                    else:
                        nc.vector.tensor_tensor(
                            out=osl,
                            in0=pov[:, :, :, 0:128],
                            in1=rden[:, 4 * k : 4 * k + 4]
                            .rearrange("p (h x) -> p h x", h=2)
                            .unsqueeze(3)
                            .broadcast_to([128, 2, 2, C]),
                            op=ALU.mult,
                        )
            odst = out[j].rearrange("(p t) c -> p t c", t=TQ)
            for qq in range(4 * hb, 4 * hb + 4):
                nc.sync.dma_start(
                    out=odst[:, 4 * qq : 4 * (qq + 1), :],
                    in_=o_sb[:, 4 * qq : 4 * (qq + 1), :],
                )
            if hb == 1:
                state.pop(j)

        def prep_all():
            with tc.tile_pool(name="prep_ps", bufs=1, space="PSUM") as prep_ps:
                # Wk' = g1 (.) Wk  (gamma1 fold)
                wkg = singles.tile([128, 2, C], F32, tag="wkg")
                for h in range(2):
                    nc.vector.tensor_scalar_mul(
                        out=wkg[:, h, :], in0=wk_sb[:, h, :], scalar1=g1c[:, h : h + 1]
                    )
                # tk[c] = sum_e b1[e] Wk[e,c]  (beta1 fold)
                ps_tk = prep_ps.tile([C, 1], F32, tag="ps_tk")
                for h in range(2):
                    nc.tensor.matmul(
                        ps_tk, wk_sb[:, h, :], b1c[:, h : h + 1],
                        start=(h == 0), stop=(h == 1),
                    )
                tk_col = singles.tile([C, 1], F32, tag="tk")
                nc.vector.tensor_copy(tk_col, ps_tk)

                # ---- key/value rows: LN(key), transposes ----
                kv_sb = singles.tile([128, 8, E], F32, tag="kv")   # t<4: key, t>=4: value
                nc.sync.dma_start(
                    out=kv_sb[:, 0:4, :],
                    in_=key[:, :, :].flatten_outer_dims().rearrange("(t p) e -> p t e", p=128),
                )
                nc.sync.dma_start(
                    out=kv_sb[:, 4:8, :],
                    in_=value[:, :, :].flatten_outer_dims().rearrange("(t p) e -> p t e", p=128),
                )
                kst = singles.tile([128, 4, 6], F32, tag="kst")
                for t in range(4):
                    nc.vector.bn_stats(out=kst[:, t, :], in_=kv_sb[:, t, :])
                kmv = singles.tile([128, 4, 2], F32, tag="kmv")
                for t in range(4):
                    nc.vector.bn_aggr(out=kmv[:, t, :], in_=kst[:, t, :])
                klnv = singles.tile([128, 4], F32, tag="klnv")
                nc.scalar.activation(
                    out=klnv, in_=kmv[:, :, 1], func=AF.Ln, bias=eps_col, scale=1.0
                )
                krs = singles.tile([128, 4], F32, tag="krs")
                nc.scalar.activation(out=krs, in_=klnv, func=AF.Exp, bias=0.0, scale=-0.5)
                kln = singles.tile([128, 4, E], F32, tag="kln")
                for t in range(4):
                    nc.gpsimd.tensor_scalar(
                        out=kln[:, t, :], in0=kv_sb[:, t, :],
                        scalar1=kmv[:, t, 0:1], scalar2=krs[:, t : t + 1],
                        op0=ALU.subtract, op1=ALU.mult,
                    )
                # transposes: [128 rows, 128 e] -> [128 e, 128 rows]
                klnT = singles.tile([128, 8, 128], F32, tag="klnT")  # u = 2t+h
                valT = singles.tile([128, 8, 128], F32, tag="valT")
                for which in range(2):  # 0: key, 1: value
                    for grp in range(2):  # t pairs (0,1) then (2,3)
                        ps_tr = prep_ps.tile([128, 4, 128], F32, tag="ps_tr")
                        for i in range(2):
                            t = grp * 2 + i
                            for h in range(2):
                                src_ap = (
                                    kln[:, t, 128 * h : 128 * (h + 1)]
                                    if which == 0
                                    else kv_sb[:, 4 + t, 128 * h : 128 * (h + 1)]
                                )
                                nc.tensor.transpose(ps_tr[:, 2 * i + h, :], src_ap, ident)
                        dst = klnT if which == 0 else valT
                        nc.scalar.copy(out=dst[:, 4 * grp : 4 * grp + 4, :], in_=ps_tr)

                # ---- per-batch projections ----
                for j in range(BPC):
                    u0 = 2 * (j // 2)
                    msl = slice(64 * (j % 2), 64 * (j % 2) + 64)
                    ps_k = prep_ps.tile([C, M], F32, tag="ps_k")
                    for h in range(2):
                        nc.tensor.matmul(
                            ps_k, wkg[:, h, :], klnT[:, u0 + h, msl],
                            start=(h == 0), stop=(h == 1),
                        )
                    kp_sb = singles.tile([C, M], F32, tag=f"kp{j}")
                    nc.vector.tensor_scalar_add(out=kp_sb, in0=ps_k, scalar1=tk_col)
                    nc.scalar.activation(
                        out=kx_all[:, j, :], in_=kp_sb, func=AF.Copy,
                        bias=0.0, scale=g2c,
                    )
                    ps_t = prep_ps.tile([128, 1], F32, tag="ps_t")
                    nc.tensor.matmul(ps_t[0:M, :], kp_sb, b2c, start=True, stop=True)
                    nc.tensor.matmul(
                        ps_t[M : 2 * M, :], kp_sb, b2c, start=True, stop=True,
                        tile_position=(0, 64), skip_group_check=True,
                    )
                    nc.scalar.activation(
                        out=tcol_all[:, j : j + 1], in_=ps_t, func=AF.Copy,
                        bias=EXP_SHIFT, scale=1.0,
                    )
                    ps_v = prep_ps.tile([128, C], F32, tag="ps_v")
                    for h in range(2):
                        nc.tensor.matmul(
                            ps_v[0:M, :], valT[:, u0 + h, msl], wv_sb[:, h, :],
                            start=(h == 0), stop=(h == 1),
                        )
                    for h in range(2):
                        nc.tensor.matmul(
                            ps_v[M : 2 * M, :], valT[:, u0 + h, msl], wv_sb[:, h, :],
                            start=(h == 0), stop=(h == 1),
                            tile_position=(0, 64), skip_group_check=True,
                        )
                    nc.scalar.copy(out=v2_all[:, j, 0:C], in_=ps_v)
                nc.vector.memset(v2_all[:, :, C : C + 1], 1.0)

        steps = 2 * BPC
        SKEW = 3
        prep_all()
        ps_qt = ctx.enter_context(tc.tile_pool(name="ps_qt", bufs=2, space="PSUM"))
        ps_st = ctx.enter_context(tc.tile_pool(name="ps_st", bufs=2, space="PSUM"))
        ps_ou = ctx.enter_context(tc.tile_pool(name="ps_ou", bufs=2, space="PSUM"))
        for st in range(0, steps + SKEW):
            if st < steps:
                phase_a(st // 2, st % 2)
            if st >= SKEW:
                pj = st - SKEW
                phase_b(pj // 2, pj % 2)

    if split:
        _split_waits(nc, limit=1)
    return nc


_NC = None


def kernel(**inputs):
    global _NC
    if _NC is None:
        _NC = _build_nc()
    q = np.ascontiguousarray(np.asarray(inputs["query"], dtype=np.float32))
    k = np.ascontiguousarray(np.asarray(inputs["key"], dtype=np.float32))
    v = np.ascontiguousarray(np.asarray(inputs["value"], dtype=np.float32))
    shared = {
        "wk": np.ascontiguousarray(np.asarray(inputs["k_proj_weight"], np.float32)),
        "wv": np.ascontiguousarray(np.asarray(inputs["v_proj_weight"], np.float32)),
        "g1": np.ascontiguousarray(np.asarray(inputs["norm1_gamma"], np.float32)),
        "b1": np.ascontiguousarray(np.asarray(inputs["norm1_beta"], np.float32)),
        "g2": np.ascontiguousarray(np.asarray(inputs["norm2_gamma"], np.float32)),
        "b2": np.ascontiguousarray(np.asarray(inputs["norm2_beta"], np.float32)),
    }
    in_maps = []
    for c in range(NCORES):
        sl = slice(c * BPC, (c + 1) * BPC)
        in_maps.append({"query": q[sl], "key": k[sl], "value": v[sl], **shared})
    res = run_bass_kernel_spmd(_NC, in_maps, core_ids=list(range(NCORES)))
    return np.concatenate([res.results[i]["out"] for i in range(NCORES)], axis=0)



# revision 60
# speedup vs baseline: 1.0473x; 1.0222x over previous
"""AttnDecoderBlock on 8 TRN2 NeuronCores — data-parallel over batch.

Per batch b:
  k   = LN_E(key[b]) ; kp = einsum('me,ec->cm', k, Wk)
  q   = LN_C(query[b])
  att = softmax(q @ kp, axis=-1)
  out = att @ (value[b] @ Wv)

Sharding: batch 64 -> 8 cores x 8 batches each. Weights/params replicated.

Per-core pipeline (row-major orientation, query rows on partitions):
  - query row r of a batch lives at partition r//32, tile r%32, so every DMA
    moves 16KB contiguous per partition (max DMA efficiency).
  - LN stats: per-tile bn_stats + batched even/odd Chan combine.
  - rsqrt(var) as Exp(-0.5*Ln(var+eps)) so the only ACT table set used is
    natural_log_exp_and_others (Ln/Exp/Copy/Identity) — no table thrash.
  - LN apply on GPSIMD (tensor_scalar), freeing DVE/ACT.
  - PE transposes q_ln tiles; ACT copies PSUM->SBUF casting to fp32r.
  - scores^T = Kx^T @ qlnT in fp32r (full rate at 512 moving columns).
  - softmax exp on ACT, bias = beta2-fold t_m - 15 (shift cancels in the
    normalize), output fp16 (fits after shift; 8x finer mantissa than bf16).
  - out tile = E_t @ [Vproj | 1] in fp16; denominator lands as column 128.
  - normalize on DVE: reciprocal + broadcast tensor_tensor from PSUM.
LN gamma/beta folds: g1 into Wk, b1 into tk column, g2 into Kx, b2 into the
exp bias t_m. value path has no LN.
"""

import numpy as np

import concourse.bass as bass
import concourse.mybir as mybir
import concourse.tile as tile
from concourse.bass_utils import run_bass_kernel_spmd
from concourse.masks import make_identity

B, N, M, E, C = 64, 4096, 64, 256, 128
NCORES = 8
BPC = B // NCORES          # batches per core
TQ = N // 128              # 32 row-tiles per batch
QG = TQ // 4               # 8 quads per batch
EPS = 1e-5
EXP_SHIFT = -15.0          # softmax shift; cancels in normalization
F32 = mybir.dt.float32
F16 = mybir.dt.float16

_ctr = [0]


def _split_waits(nc, limit=1):
    """The axon-path walrus accepts only `limit` sem-waits per instruction;
    move excess onto preceding same-engine NOPs (program order on the engine
    makes this equivalent)."""
    for f in nc.m.functions:
        for bb in f.blocks:
            out = []
            changed = False
            for inst in bb.instructions:
                si = inst.sync_info
                if si is not None and si.on_wait is not None and len(si.on_wait) > limit:
                    waits = list(si.on_wait)
                    while len(waits) > limit:
                        chunk, waits = waits[:limit], waits[limit:]
                        _ctr[0] += 1
                        nop = mybir.InstNoOp(name=f"I-wsplit-{_ctr[0]}", ins=[], outs=[])
                        nop.engine = inst.engine
                        nop.sync_info = mybir.SyncInfo(on_wait=chunk, on_update=[])
                        out.append(nop)
                        changed = True
                    inst.sync_info = mybir.SyncInfo(on_wait=waits, on_update=si.on_update)
                out.append(inst)
            if changed:
                bb.instructions = out
    return nc


def _build_nc(split=True):
    nc = bass.Bass()
    AF = mybir.ActivationFunctionType
    ALU = mybir.AluOpType

    query = nc.dram_tensor("query", [BPC, N, C], F32, kind="ExternalInput")
    key = nc.dram_tensor("key", [BPC, M, E], F32, kind="ExternalInput")
    value = nc.dram_tensor("value", [BPC, M, E], F32, kind="ExternalInput")
    wk = nc.dram_tensor("wk", [E, C], F32, kind="ExternalInput")
    wv = nc.dram_tensor("wv", [E, C], F32, kind="ExternalInput")
    g1 = nc.dram_tensor("g1", [E], F32, kind="ExternalInput")
    b1 = nc.dram_tensor("b1", [E], F32, kind="ExternalInput")
    g2 = nc.dram_tensor("g2", [C], F32, kind="ExternalInput")
    b2 = nc.dram_tensor("b2", [C], F32, kind="ExternalInput")
    out = nc.dram_tensor("out", [BPC, N, C], F32, kind="ExternalOutput")

    import contextlib
    with tile.TileContext(nc) as tc, contextlib.ExitStack() as ctx:
        singles = ctx.enter_context(tc.tile_pool(name="singles", bufs=1))

        ident = singles.tile([128, 128], F32, tag="ident")
        make_identity(nc, ident)
        ident16 = singles.tile([128, 128], F16, tag="ident16")
        nc.gpsimd.tensor_copy(ident16, ident)
        eps_col = singles.tile([128, 1], F32, tag="eps")
        nc.vector.memset(eps_col, EPS)

        # ---- weights / params ----
        wk_sb = singles.tile([128, 2, C], F32, tag="wk")       # [p, h, c], e=128h+p
        wv_sb = singles.tile([128, 2, C], F32, tag="wv")
        nc.sync.dma_start(out=wk_sb, in_=wk.rearrange("(h p) c -> p h c", p=128))
        nc.sync.dma_start(out=wv_sb, in_=wv.rearrange("(h p) c -> p h c", p=128))
        g1c = singles.tile([128, 2], F32, tag="g1")
        b1c = singles.tile([128, 2], F32, tag="b1")
        nc.sync.dma_start(out=g1c, in_=g1.rearrange("(h p) -> p h", p=128))
        nc.sync.dma_start(out=b1c, in_=b1.rearrange("(h p) -> p h", p=128))
        g2c = singles.tile([128, 1], F32, tag="g2")
        b2c = singles.tile([128, 1], F32, tag="b2")
        nc.sync.dma_start(out=g2c, in_=g2[:].unsqueeze(1))
        nc.sync.dma_start(out=b2c, in_=b2[:].unsqueeze(1))

        kx_all = singles.tile([128, BPC, M], F16, tag="kx")     # g2 (.) KP
        tcol_all = singles.tile([128, BPC], F32, tag="tcol")    # t_m + shift, dup rows
        v2_all = singles.tile([128, BPC, C + 1], F16, tag="v2") # [Vproj | 1], dup rows

        # ---- main loop ----
        qpool = ctx.enter_context(tc.tile_pool(name="qpool", bufs=4))
        opool = ctx.enter_context(tc.tile_pool(name="opool", bufs=4))
        small = ctx.enter_context(tc.tile_pool(name="small", bufs=4))
        mid = ctx.enter_context(tc.tile_pool(name="mid", bufs=6))

        state = {}
        H = TQ // 2

        def phase_a(j, hb):
            if hb == 0:
                q_sb = qpool.tile([128, TQ, C], F32, tag="q")
                qst = small.tile([128, TQ, 6], F32, tag="qst")
                mu = small.tile([128, TQ], F32, tag="mu")
                qr = small.tile([128, TQ], F32, tag="qr")
                dlt = small.tile([128, TQ], F32, tag="dlt")
                v128 = small.tile([128, TQ], F32, tag="v128")
                qlv = small.tile([128, TQ], F32, tag="qlv")
                qln = qpool.tile([128, TQ, C], F16, tag="qln")
                o_sb = opool.tile([128, TQ, C], F32, tag="o")
                state[j] = (q_sb, qst, mu, qr, dlt, v128, qlv, qln, o_sb)
            q_sb, qst, mu, qr, dlt, v128, qlv, qln, o_sb = state[j]
            qsrc = query[j].rearrange("(p t) c -> p t c", t=TQ)
            for dd in range(2 * hb, 2 * hb + 2):
                nc.sync.dma_start(
                    out=q_sb[:, 8 * dd : 8 * (dd + 1), :],
                    in_=qsrc[:, 8 * dd : 8 * (dd + 1), :],
                )
            sl = slice(hb * H, (hb + 1) * H)
            for t in range(hb * H, (hb + 1) * H):
                nc.vector.bn_stats(out=qst[:, t, :], in_=q_sb[:, t, :])
            me, m2e = qst[:, sl, 1], qst[:, sl, 2]
            mo, m2o = qst[:, sl, 4], qst[:, sl, 5]
            nc.vector.tensor_tensor(out=mu[:, sl], in0=me, in1=mo, op=ALU.add)
            nc.vector.tensor_scalar_mul(out=mu[:, sl], in0=mu[:, sl], scalar1=0.5)
            nc.vector.tensor_tensor(out=dlt[:, sl], in0=me, in1=mo, op=ALU.subtract)
            nc.vector.tensor_tensor(out=dlt[:, sl], in0=dlt[:, sl], in1=dlt[:, sl], op=ALU.mult)
            nc.vector.tensor_tensor(out=v128[:, sl], in0=m2e, in1=m2o, op=ALU.add)
            nc.gpsimd.scalar_tensor_tensor(
                out=v128[:, sl], in0=dlt[:, sl], scalar=32.0, in1=v128[:, sl],
                op0=ALU.mult, op1=ALU.add,
            )
            nc.scalar.activation(
                out=qlv[:, sl], in_=v128[:, sl], func=AF.Ln, bias=eps_col, scale=1.0 / C
            )
            nc.scalar.activation(
                out=qr[:, sl], in_=qlv[:, sl], func=AF.Exp, bias=0.0, scale=-0.5
            )
            apply_eng = nc.vector if j == 0 else nc.gpsimd
            for t in range(hb * H, (hb + 1) * H):
                apply_eng.tensor_scalar(
                    out=qln[:, t, :], in0=q_sb[:, t, :],
                    scalar1=mu[:, t : t + 1], scalar2=qr[:, t : t + 1],
                    op0=ALU.subtract, op1=ALU.mult,
                )

        def phase_b(j, hb):
            q_sb, qst, mu, qr, dlt, v128, qlv, qln, o_sb = state[j]
            for g2 in range(hb * (QG // 4), (hb + 1) * (QG // 4)):
                p_qt = ps_qt.tile([128, 8, 128], F16, tag="p_qt")
                for i in range(8):
                    nc.tensor.transpose(
                        p_qt[:, i, :], qln[:, 8 * g2 + i, :], ident16
                    )
                qlnT = mid.tile([128, 2, 512], F16, tag="qlnT")
                for half in range(2):
                    src = p_qt[:, 4 * half : 4 * half + 4, :].rearrange(
                        "p a b -> p (a b)"
                    )
                    nc.scalar.copy(out=qlnT[:, half, :], in_=src)
                p_st = ps_st.tile([128, 512], F32, tag="p_st")
                nc.tensor.matmul(
                    p_st[0:M, :], kx_all[:, j, :], qlnT[:, 0, :],
                    start=True, stop=True,
                )
                nc.tensor.matmul(
                    p_st[M:128, :], kx_all[:, j, :], qlnT[:, 1, :],
                    start=True, stop=True,
                    tile_position=(0, 64), skip_group_check=True,
                )
                et = mid.tile([128, 512], F16, tag="et")
                nc.scalar.activation(
                    out=et, in_=p_st, func=AF.Exp,
                    bias=tcol_all[:, j : j + 1], scale=1.0,
                )
                rden = small.tile([128, 8], F32, tag="rden")
                for k in range(2):
                    p_o = ps_ou.tile([128, 2, 512], F32, tag="p_o")
                    for half in range(2):
                        pb = M * half
                        for i in range(2):
                            t = 2 * k + i
                            nc.tensor.matmul(
                                p_o[:, half, 132 * i + 128 : 132 * i + 129],
                                et[pb : pb + M, 128 * t : 128 * (t + 1)],
                                v2_all[pb : pb + M, j, C : C + 1],
                                start=True, stop=True,
                                tile_position=(pb, 0), skip_group_check=True,
                            )
                    pov = p_o[:, :, 0:264].rearrange("p h (x c) -> p h x c", x=2)
                    nc.vector.reciprocal(
                        out=rden[:, 4 * k : 4 * k + 4],
                        in_=pov[:, :, :, 128],
                    )
                    for half in range(2):
                        pb = M * half
                        for i in range(2):
                            t = 2 * k + i
                            nc.tensor.matmul(
                                p_o[:, half, 132 * i : 132 * i + 128],
                                et[pb : pb + M, 128 * t : 128 * (t + 1)],
                                v2_all[pb : pb + M, j, 0:C],
                                start=True, stop=True,
                                tile_position=(pb, 0), skip_group_check=True,
                            )
                    osl = (
                        o_sb[:, 8 * g2 : 8 * g2 + 8, :]
                        .rearrange("p (h x) c -> p h x c", h=2)[:, :, 2 * k : 2 * k + 2, :]
                    )
                    if (j * (QG // 2) + g2) % 2 == 1:
                        for hh in range(2):
                            for xx in range(2):
                                nc.scalar.activation(
                                    out=osl[:, hh, xx, :],
                                    in_=pov[:, hh, xx, 0:128],
                                    func=AF.Copy, bias=0.0,
                                    scale=rden[:, 4 * k + 2 * hh + xx : 4 * k + 2 * hh + xx + 1],
                                )
                    else:
                        nc.vector.tensor_tensor(
                            out=osl,
                            in0=pov[:, :, :, 0:128],
                            in1=rden[:, 4 * k : 4 * k + 4]
                            .rearrange("p (h x) -> p h x", h=2)
                            .unsqueeze(3)
                            .broadcast_to([128, 2, 2, C]),
                            op=ALU.mult,
                        )
            odst = out[j].rearrange("(p t) c -> p t c", t=TQ)
            for qq in range(4 * hb, 4 * hb + 4):
                nc.sync.dma_start(
                    out=odst[:, 4 * qq : 4 * (qq + 1), :],
                    in_=o_sb[:, 4 * qq : 4 * (qq + 1), :],
                )
            if hb == 1:
                state.pop(j)

        def prep_all():
            with tc.tile_pool(name="prep_ps", bufs=1, space="PSUM") as prep_ps:
                # Wk' = g1 (.) Wk  (gamma1 fold)
                wkg = singles.tile([128, 2, C], F32, tag="wkg")
                for h in range(2):
                    nc.vector.tensor_scalar_mul(
                        out=wkg[:, h, :], in0=wk_sb[:, h, :], scalar1=g1c[:, h : h + 1]
                    )
                # tk[c] = sum_e b1[e] Wk[e,c]  (beta1 fold)
                ps_tk = prep_ps.tile([C, 1], F32, tag="ps_tk")
                for h in range(2):
                    nc.tensor.matmul(
                        ps_tk, wk_sb[:, h, :], b1c[:, h : h + 1],
                        start=(h == 0), stop=(h == 1),
                    )
                tk_col = singles.tile([C, 1], F32, tag="tk")
                nc.vector.tensor_copy(tk_col, ps_tk)

                # ---- key/value rows: LN(key), transposes ----
                kv_sb = singles.tile([128, 8, E], F32, tag="kv")   # t<4: key, t>=4: value
                nc.sync.dma_start(
                    out=kv_sb[:, 0:4, :],
                    in_=key[:, :, :].flatten_outer_dims().rearrange("(t p) e -> p t e", p=128),
                )
                nc.sync.dma_start(
                    out=kv_sb[:, 4:8, :],
                    in_=value[:, :, :].flatten_outer_dims().rearrange("(t p) e -> p t e", p=128),
                )
                kst = singles.tile([128, 4, 6], F32, tag="kst")
                for t in range(4):
                    nc.vector.bn_stats(out=kst[:, t, :], in_=kv_sb[:, t, :])
                kmv = singles.tile([128, 4, 2], F32, tag="kmv")
                for t in range(4):
                    nc.vector.bn_aggr(out=kmv[:, t, :], in_=kst[:, t, :])
                klnv = singles.tile([128, 4], F32, tag="klnv")
                nc.scalar.activation(
                    out=klnv, in_=kmv[:, :, 1], func=AF.Ln, bias=eps_col, scale=1.0
                )
                krs = singles.tile([128, 4], F32, tag="krs")
                nc.scalar.activation(out=krs, in_=klnv, func=AF.Exp, bias=0.0, scale=-0.5)
                kln = singles.tile([128, 4, E], F32, tag="kln")
                for t in range(4):
                    nc.gpsimd.tensor_scalar(
                        out=kln[:, t, :], in0=kv_sb[:, t, :],
                        scalar1=kmv[:, t, 0:1], scalar2=krs[:, t : t + 1],
                        op0=ALU.subtract, op1=ALU.mult,
                    )
                # transposes: [128 rows, 128 e] -> [128 e, 128 rows]
                klnT = singles.tile([128, 8, 128], F32, tag="klnT")  # u = 2t+h
                valT = singles.tile([128, 8, 128], F32, tag="valT")
                for which in range(2):  # 0: key, 1: value
                    for grp in range(2):  # t pairs (0,1) then (2,3)
                        ps_tr = prep_ps.tile([128, 4, 128], F32, tag="ps_tr")
                        for i in range(2):
                            t = grp * 2 + i
                            for h in range(2):
                                src_ap = (
                                    kln[:, t, 128 * h : 128 * (h + 1)]
                                    if which == 0
                                    else kv_sb[:, 4 + t, 128 * h : 128 * (h + 1)]
                                )
                                nc.tensor.transpose(ps_tr[:, 2 * i + h, :], src_ap, ident)
                        dst = klnT if which == 0 else valT
                        nc.scalar.copy(out=dst[:, 4 * grp : 4 * grp + 4, :], in_=ps_tr)

                # ---- per-batch projections ----
                for j in range(BPC):
                    u0 = 2 * (j // 2)
                    msl = slice(64 * (j % 2), 64 * (j % 2) + 64)
                    ps_k = prep_ps.tile([C, M], F32, tag="ps_k")
                    for h in range(2):
                        nc.tensor.matmul(
                            ps_k, wkg[:, h, :], klnT[:, u0 + h, msl],
                            start=(h == 0), stop=(h == 1),
                        )
                    kp_sb = singles.tile([C, M], F32, tag=f"kp{j}")
                    nc.vector.tensor_scalar_add(out=kp_sb, in0=ps_k, scalar1=tk_col)
                    nc.scalar.activation(
                        out=kx_all[:, j, :], in_=kp_sb, func=AF.Copy,
                        bias=0.0, scale=g2c,
                    )
                    ps_t = prep_ps.tile([128, 1], F32, tag="ps_t")
                    nc.tensor.matmul(ps_t[0:M, :], kp_sb, b2c, start=True, stop=True)
                    nc.tensor.matmul(
                        ps_t[M : 2 * M, :], kp_sb, b2c, start=True, stop=True,
                        tile_position=(0, 64), skip_group_check=True,
                    )
                    nc.scalar.activation(
                        out=tcol_all[:, j : j + 1], in_=ps_t, func=AF.Copy,
                        bias=EXP_SHIFT, scale=1.0,
                    )
                    ps_v = prep_ps.tile([128, C], F32, tag="ps_v")
                    for h in range(2):
                        nc.tensor.matmul(
                            ps_v[0:M, :], valT[:, u0 + h, msl], wv_sb[:, h, :],
                            start=(h == 0), stop=(h == 1),
                        )
                    for h in range(2):
                        nc.tensor.matmul(
                            ps_v[M : 2 * M, :], valT[:, u0 + h, msl], wv_sb[:, h, :],
                            start=(h == 0), stop=(h == 1),
                            tile_position=(0, 64), skip_group_check=True,
                        )
                    nc.scalar.copy(out=v2_all[:, j, 0:C], in_=ps_v)
                nc.vector.memset(v2_all[:, :, C : C + 1], 1.0)

        steps = 2 * BPC
        SKEW = 3
        prep_all()
        ps_qt = ctx.enter_context(tc.tile_pool(name="ps_qt", bufs=2, space="PSUM"))
        ps_st = ctx.enter_context(tc.tile_pool(name="ps_st", bufs=2, space="PSUM"))
        ps_ou = ctx.enter_context(tc.tile_pool(name="ps_ou", bufs=2, space="PSUM"))
        for st in range(0, steps + SKEW):
            if st < steps:
                phase_a(st // 2, st % 2)
            if st >= SKEW:
                pj = st - SKEW
                phase_b(pj // 2, pj % 2)

    if split:
        _split_waits(nc, limit=1)
    return nc


_NC = None


def kernel(**inputs):
    global _NC
    if _NC is None:
        _NC = _build_nc()
    q = np.ascontiguousarray(np.asarray(inputs["query"], dtype=np.float32))
    k = np.ascontiguousarray(np.asarray(inputs["key"], dtype=np.float32))
    v = np.ascontiguousarray(np.asarray(inputs["value"], dtype=np.float32))
    shared = {
        "wk": np.ascontiguousarray(np.asarray(inputs["k_proj_weight"], np.float32)),
        "wv": np.ascontiguousarray(np.asarray(inputs["v_proj_weight"], np.float32)),
        "g1": np.ascontiguousarray(np.asarray(inputs["norm1_gamma"], np.float32)),
        "b1": np.ascontiguousarray(np.asarray(inputs["norm1_beta"], np.float32)),
        "g2": np.ascontiguousarray(np.asarray(inputs["norm2_gamma"], np.float32)),
        "b2": np.ascontiguousarray(np.asarray(inputs["norm2_beta"], np.float32)),
    }
    in_maps = []
    for c in range(NCORES):
        sl = slice(c * BPC, (c + 1) * BPC)
        in_maps.append({"query": q[sl], "key": k[sl], "value": v[sl], **shared})
    res = run_bass_kernel_spmd(_NC, in_maps, core_ids=list(range(NCORES)))
    return np.concatenate([res.results[i]["out"] for i in range(NCORES)], axis=0)



# revision 65
# speedup vs baseline: 1.0538x; 1.0062x over previous
"""AttnDecoderBlock on 8 TRN2 NeuronCores — data-parallel over batch.

Per batch b:
  k   = LN_E(key[b]) ; kp = einsum('me,ec->cm', k, Wk)
  q   = LN_C(query[b])
  att = softmax(q @ kp, axis=-1)
  out = att @ (value[b] @ Wv)

Sharding: batch 64 -> 8 cores x 8 batches each. Weights/params replicated.

Per-core pipeline (row-major orientation, query rows on partitions):
  - query row r of a batch lives at partition r//32, tile r%32, so every DMA
    moves 16KB contiguous per partition (max DMA efficiency).
  - LN stats: per-tile bn_stats + batched even/odd Chan combine.
  - rsqrt(var) as Exp(-0.5*Ln(var+eps)) so the only ACT table set used is
    natural_log_exp_and_others (Ln/Exp/Copy/Identity) — no table thrash.
  - LN apply on GPSIMD (tensor_scalar), freeing DVE/ACT.
  - PE transposes q_ln tiles; ACT copies PSUM->SBUF casting to fp32r.
  - scores^T = Kx^T @ qlnT in fp32r (full rate at 512 moving columns).
  - softmax exp on ACT, bias = beta2-fold t_m - 15 (shift cancels in the
    normalize), output fp16 (fits after shift; 8x finer mantissa than bf16).
  - out tile = E_t @ [Vproj | 1] in fp16; denominator lands as column 128.
  - normalize on DVE: reciprocal + broadcast tensor_tensor from PSUM.
LN gamma/beta folds: g1 into Wk, b1 into tk column, g2 into Kx, b2 into the
exp bias t_m. value path has no LN.
"""

import numpy as np

import concourse.bass as bass
import concourse.mybir as mybir
import concourse.tile as tile
from concourse.bass_utils import run_bass_kernel_spmd
from concourse.masks import make_identity

B, N, M, E, C = 64, 4096, 64, 256, 128
NCORES = 8
BPC = B // NCORES          # batches per core
TQ = N // 128              # 32 row-tiles per batch
QG = TQ // 4               # 8 quads per batch
EPS = 1e-5
EXP_SHIFT = -15.0          # softmax shift; cancels in normalization
F32 = mybir.dt.float32
F16 = mybir.dt.float16

_ctr = [0]


def _split_waits(nc, limit=1):
    """The axon-path walrus accepts only `limit` sem-waits per instruction;
    move excess onto preceding same-engine NOPs (program order on the engine
    makes this equivalent)."""
    for f in nc.m.functions:
        for bb in f.blocks:
            out = []
            changed = False
            for inst in bb.instructions:
                si = inst.sync_info
                if si is not None and si.on_wait is not None and len(si.on_wait) > limit:
                    waits = list(si.on_wait)
                    while len(waits) > limit:
                        chunk, waits = waits[:limit], waits[limit:]
                        _ctr[0] += 1
                        nop = mybir.InstNoOp(name=f"I-wsplit-{_ctr[0]}", ins=[], outs=[])
                        nop.engine = inst.engine
                        nop.sync_info = mybir.SyncInfo(on_wait=chunk, on_update=[])
                        out.append(nop)
                        changed = True
                    inst.sync_info = mybir.SyncInfo(on_wait=waits, on_update=si.on_update)
                out.append(inst)
            if changed:
                bb.instructions = out
    return nc


def _build_nc(split=True):
    nc = bass.Bass()
    AF = mybir.ActivationFunctionType
    ALU = mybir.AluOpType

    query = nc.dram_tensor("query", [BPC, N, C], F32, kind="ExternalInput")
    key = nc.dram_tensor("key", [BPC, M, E], F32, kind="ExternalInput")
    value = nc.dram_tensor("value", [BPC, M, E], F32, kind="ExternalInput")
    wk = nc.dram_tensor("wk", [E, C], F32, kind="ExternalInput")
    wv = nc.dram_tensor("wv", [E, C], F32, kind="ExternalInput")
    g1 = nc.dram_tensor("g1", [E], F32, kind="ExternalInput")
    b1 = nc.dram_tensor("b1", [E], F32, kind="ExternalInput")
    g2 = nc.dram_tensor("g2", [C], F32, kind="ExternalInput")
    b2 = nc.dram_tensor("b2", [C], F32, kind="ExternalInput")
    out = nc.dram_tensor("out", [BPC, N, C], F32, kind="ExternalOutput")

    import contextlib
    with tile.TileContext(nc) as tc, contextlib.ExitStack() as ctx:
        singles = ctx.enter_context(tc.tile_pool(name="singles", bufs=1))

        ident = singles.tile([128, 128], F32, tag="ident")
        make_identity(nc, ident)
        ident16 = singles.tile([128, 128], F16, tag="ident16")
        nc.gpsimd.tensor_copy(ident16, ident)
        eps_col = singles.tile([128, 1], F32, tag="eps")
        nc.vector.memset(eps_col, EPS)

        # ---- weights / params ----
        wk_sb = singles.tile([128, 2, C], F32, tag="wk")       # [p, h, c], e=128h+p
        wv_sb = singles.tile([128, 2, C], F32, tag="wv")
        nc.sync.dma_start(out=wk_sb, in_=wk.rearrange("(h p) c -> p h c", p=128))
        nc.sync.dma_start(out=wv_sb, in_=wv.rearrange("(h p) c -> p h c", p=128))
        g1c = singles.tile([128, 2], F32, tag="g1")
        b1c = singles.tile([128, 2], F32, tag="b1")
        nc.sync.dma_start(out=g1c, in_=g1.rearrange("(h p) -> p h", p=128))
        nc.sync.dma_start(out=b1c, in_=b1.rearrange("(h p) -> p h", p=128))
        g2c = singles.tile([128, 1], F32, tag="g2")
        b2c = singles.tile([128, 1], F32, tag="b2")
        nc.sync.dma_start(out=g2c, in_=g2[:].unsqueeze(1))
        nc.sync.dma_start(out=b2c, in_=b2[:].unsqueeze(1))

        kx_all = singles.tile([128, BPC, M], F16, tag="kx")     # g2 (.) KP
        tcol_all = singles.tile([128, BPC], F32, tag="tcol")    # t_m + shift, dup rows
        v2_all = singles.tile([128, BPC, C + 1], F16, tag="v2") # [Vproj | 1], dup rows

        # ---- main loop ----
        qpool = ctx.enter_context(tc.tile_pool(name="qpool", bufs=4))
        opool = ctx.enter_context(tc.tile_pool(name="opool", bufs=4))
        small = ctx.enter_context(tc.tile_pool(name="small", bufs=4))
        mid = ctx.enter_context(tc.tile_pool(name="mid", bufs=6))

        state = {}
        H = TQ // 2

        def phase_a(j, hb):
            if hb == 0:
                q_sb = qpool.tile([128, TQ, C], F32, tag="q")
                qst = small.tile([128, TQ, 6], F32, tag="qst")
                mu = small.tile([128, TQ], F32, tag="mu")
                qr = small.tile([128, TQ], F32, tag="qr")
                dlt = small.tile([128, TQ], F32, tag="dlt")
                v128 = small.tile([128, TQ], F32, tag="v128")
                qlv = small.tile([128, TQ], F32, tag="qlv")
                qln = qpool.tile([128, TQ, C], F16, tag="qln")
                o_sb = opool.tile([128, TQ, C], F32, tag="o")
                state[j] = (q_sb, qst, mu, qr, dlt, v128, qlv, qln, o_sb)
            q_sb, qst, mu, qr, dlt, v128, qlv, qln, o_sb = state[j]
            qsrc = query[j].rearrange("(p t) c -> p t c", t=TQ)
            if hb == 0:
                for dd in range(4):
                    nc.sync.dma_start(
                        out=q_sb[:, 8 * dd : 8 * (dd + 1), :],
                        in_=qsrc[:, 8 * dd : 8 * (dd + 1), :],
                    )
            sl = slice(hb * H, (hb + 1) * H)
            for t in range(hb * H, (hb + 1) * H):
                nc.vector.bn_stats(out=qst[:, t, :], in_=q_sb[:, t, :])
            me, m2e = qst[:, sl, 1], qst[:, sl, 2]
            mo, m2o = qst[:, sl, 4], qst[:, sl, 5]
            nc.vector.tensor_tensor(out=mu[:, sl], in0=me, in1=mo, op=ALU.add)
            nc.vector.tensor_scalar_mul(out=mu[:, sl], in0=mu[:, sl], scalar1=0.5)
            nc.vector.tensor_tensor(out=dlt[:, sl], in0=me, in1=mo, op=ALU.subtract)
            nc.vector.tensor_tensor(out=dlt[:, sl], in0=dlt[:, sl], in1=dlt[:, sl], op=ALU.mult)
            nc.vector.tensor_tensor(out=v128[:, sl], in0=m2e, in1=m2o, op=ALU.add)
            nc.gpsimd.scalar_tensor_tensor(
                out=v128[:, sl], in0=dlt[:, sl], scalar=32.0, in1=v128[:, sl],
                op0=ALU.mult, op1=ALU.add,
            )
            nc.scalar.activation(
                out=qlv[:, sl], in_=v128[:, sl], func=AF.Ln, bias=eps_col, scale=1.0 / C
            )
            nc.scalar.activation(
                out=qr[:, sl], in_=qlv[:, sl], func=AF.Exp, bias=0.0, scale=-0.5
            )
            apply_eng = nc.vector if j == 0 else nc.gpsimd
            for t in range(hb * H, (hb + 1) * H):
                apply_eng.tensor_scalar(
                    out=qln[:, t, :], in0=q_sb[:, t, :],
                    scalar1=mu[:, t : t + 1], scalar2=qr[:, t : t + 1],
                    op0=ALU.subtract, op1=ALU.mult,
                )

        def phase_b(j, hb):
            q_sb, qst, mu, qr, dlt, v128, qlv, qln, o_sb = state[j]
            for g2 in range(hb * (QG // 4), (hb + 1) * (QG // 4)):
                p_qt = ps_qt.tile([128, 8, 128], F16, tag="p_qt")
                for i in range(8):
                    nc.tensor.transpose(
                        p_qt[:, i, :], qln[:, 8 * g2 + i, :], ident16
                    )
                qlnT = mid.tile([128, 2, 512], F16, tag="qlnT")
                for half in range(2):
                    src = p_qt[:, 4 * half : 4 * half + 4, :].rearrange(
                        "p a b -> p (a b)"
                    )
                    nc.scalar.copy(out=qlnT[:, half, :], in_=src)
                p_st = ps_st.tile([128, 512], F32, tag="p_st")
                nc.tensor.matmul(
                    p_st[0:M, :], kx_all[:, j, :], qlnT[:, 0, :],
                    start=True, stop=True,
                )
                nc.tensor.matmul(
                    p_st[M:128, :], kx_all[:, j, :], qlnT[:, 1, :],
                    start=True, stop=True,
                    tile_position=(0, 64), skip_group_check=True,
                )
                et = mid.tile([128, 512], F16, tag="et")
                nc.scalar.activation(
                    out=et, in_=p_st, func=AF.Exp,
                    bias=tcol_all[:, j : j + 1], scale=1.0,
                )
                rden = small.tile([128, 8], F32, tag="rden")
                for k in range(2):
                    p_o = ps_ou.tile([128, 2, 512], F32, tag="p_o")
                    for half in range(2):
                        pb = M * half
                        for i in range(2):
                            t = 2 * k + i
                            nc.tensor.matmul(
                                p_o[:, half, 132 * i + 128 : 132 * i + 129],
                                et[pb : pb + M, 128 * t : 128 * (t + 1)],
                                v2_all[pb : pb + M, j, C : C + 1],
                                start=True, stop=True,
                                tile_position=(pb, 0), skip_group_check=True,
                            )
                    pov = p_o[:, :, 0:264].rearrange("p h (x c) -> p h x c", x=2)
                    nc.vector.reciprocal(
                        out=rden[:, 4 * k : 4 * k + 4],
                        in_=pov[:, :, :, 128],
                    )
                    for half in range(2):
                        pb = M * half
                        for i in range(2):
                            t = 2 * k + i
                            nc.tensor.matmul(
                                p_o[:, half, 132 * i : 132 * i + 128],
                                et[pb : pb + M, 128 * t : 128 * (t + 1)],
                                v2_all[pb : pb + M, j, 0:C],
                                start=True, stop=True,
                                tile_position=(pb, 0), skip_group_check=True,
                            )
                    osl = (
                        o_sb[:, 8 * g2 : 8 * g2 + 8, :]
                        .rearrange("p (h x) c -> p h x c", h=2)[:, :, 2 * k : 2 * k + 2, :]
                    )
                    if (j * (QG // 2) + g2) % 2 == 1:
                        for hh in range(2):
                            for xx in range(2):
                                nc.scalar.activation(
                                    out=osl[:, hh, xx, :],
                                    in_=pov[:, hh, xx, 0:128],
                                    func=AF.Copy, bias=0.0,
                                    scale=rden[:, 4 * k + 2 * hh + xx : 4 * k + 2 * hh + xx + 1],
                                )
                    else:
                        nc.vector.tensor_tensor(
                            out=osl,
                            in0=pov[:, :, :, 0:128],
                            in1=rden[:, 4 * k : 4 * k + 4]
                            .rearrange("p (h x) -> p h x", h=2)
                            .unsqueeze(3)
                            .broadcast_to([128, 2, 2, C]),
                            op=ALU.mult,
                        )
            odst = out[j].rearrange("(p t) c -> p t c", t=TQ)
            for qq in range(4 * hb, 4 * hb + 4):
                nc.sync.dma_start(
                    out=odst[:, 4 * qq : 4 * (qq + 1), :],
                    in_=o_sb[:, 4 * qq : 4 * (qq + 1), :],
                )
            if hb == 1:
                state.pop(j)

        def prep_all():
            with tc.tile_pool(name="prep_ps", bufs=1, space="PSUM") as prep_ps:
                # Wk' = g1 (.) Wk  (gamma1 fold)
                wkg = singles.tile([128, 2, C], F32, tag="wkg")
                for h in range(2):
                    nc.vector.tensor_scalar_mul(
                        out=wkg[:, h, :], in0=wk_sb[:, h, :], scalar1=g1c[:, h : h + 1]
                    )
                # tk[c] = sum_e b1[e] Wk[e,c]  (beta1 fold)
                ps_tk = prep_ps.tile([C, 1], F32, tag="ps_tk")
                for h in range(2):
                    nc.tensor.matmul(
                        ps_tk, wk_sb[:, h, :], b1c[:, h : h + 1],
                        start=(h == 0), stop=(h == 1),
                    )
                tk_col = singles.tile([C, 1], F32, tag="tk")
                nc.vector.tensor_copy(tk_col, ps_tk)

                # ---- key/value rows: LN(key), transposes ----
                kv_sb = singles.tile([128, 8, E], F32, tag="kv")   # t<4: key, t>=4: value
                nc.sync.dma_start(
                    out=kv_sb[:, 0:4, :],
                    in_=key[:, :, :].flatten_outer_dims().rearrange("(t p) e -> p t e", p=128),
                )
                nc.sync.dma_start(
                    out=kv_sb[:, 4:8, :],
                    in_=value[:, :, :].flatten_outer_dims().rearrange("(t p) e -> p t e", p=128),
                )
                kst = singles.tile([128, 4, 6], F32, tag="kst")
                for t in range(4):
                    nc.vector.bn_stats(out=kst[:, t, :], in_=kv_sb[:, t, :])
                kmv = singles.tile([128, 4, 2], F32, tag="kmv")
                for t in range(4):
                    nc.vector.bn_aggr(out=kmv[:, t, :], in_=kst[:, t, :])
                klnv = singles.tile([128, 4], F32, tag="klnv")
                nc.scalar.activation(
                    out=klnv, in_=kmv[:, :, 1], func=AF.Ln, bias=eps_col, scale=1.0
                )
                krs = singles.tile([128, 4], F32, tag="krs")
                nc.scalar.activation(out=krs, in_=klnv, func=AF.Exp, bias=0.0, scale=-0.5)
                kln = singles.tile([128, 4, E], F32, tag="kln")
                for t in range(4):
                    nc.gpsimd.tensor_scalar(
                        out=kln[:, t, :], in0=kv_sb[:, t, :],
                        scalar1=kmv[:, t, 0:1], scalar2=krs[:, t : t + 1],
                        op0=ALU.subtract, op1=ALU.mult,
                    )
                # transposes: [128 rows, 128 e] -> [128 e, 128 rows]
                klnT = singles.tile([128, 8, 128], F32, tag="klnT")  # u = 2t+h
                valT = singles.tile([128, 8, 128], F32, tag="valT")
                for which in range(2):  # 0: key, 1: value
                    for grp in range(2):  # t pairs (0,1) then (2,3)
                        ps_tr = prep_ps.tile([128, 4, 128], F32, tag="ps_tr")
                        for i in range(2):
                            t = grp * 2 + i
                            for h in range(2):
                                src_ap = (
                                    kln[:, t, 128 * h : 128 * (h + 1)]
                                    if which == 0
                                    else kv_sb[:, 4 + t, 128 * h : 128 * (h + 1)]
                                )
                                nc.tensor.transpose(ps_tr[:, 2 * i + h, :], src_ap, ident)
                        dst = klnT if which == 0 else valT
                        nc.scalar.copy(out=dst[:, 4 * grp : 4 * grp + 4, :], in_=ps_tr)

                # ---- per-batch projections ----
                for j in range(BPC):
                    u0 = 2 * (j // 2)
                    msl = slice(64 * (j % 2), 64 * (j % 2) + 64)
                    ps_k = prep_ps.tile([C, M], F32, tag="ps_k")
                    for h in range(2):
                        nc.tensor.matmul(
                            ps_k, wkg[:, h, :], klnT[:, u0 + h, msl],
                            start=(h == 0), stop=(h == 1),
                        )
                    kp_sb = singles.tile([C, M], F32, tag=f"kp{j}")
                    nc.vector.tensor_scalar_add(out=kp_sb, in0=ps_k, scalar1=tk_col)
                    nc.scalar.activation(
                        out=kx_all[:, j, :], in_=kp_sb, func=AF.Copy,
                        bias=0.0, scale=g2c,
                    )
                    ps_t = prep_ps.tile([128, 1], F32, tag="ps_t")
                    nc.tensor.matmul(ps_t[0:M, :], kp_sb, b2c, start=True, stop=True)
                    nc.tensor.matmul(
                        ps_t[M : 2 * M, :], kp_sb, b2c, start=True, stop=True,
                        tile_position=(0, 64), skip_group_check=True,
                    )
                    nc.scalar.activation(
                        out=tcol_all[:, j : j + 1], in_=ps_t, func=AF.Copy,
                        bias=EXP_SHIFT, scale=1.0,
                    )
                    ps_v = prep_ps.tile([128, C], F32, tag="ps_v")
                    for h in range(2):
                        nc.tensor.matmul(
                            ps_v[0:M, :], valT[:, u0 + h, msl], wv_sb[:, h, :],
                            start=(h == 0), stop=(h == 1),
                        )
                    for h in range(2):
                        nc.tensor.matmul(
                            ps_v[M : 2 * M, :], valT[:, u0 + h, msl], wv_sb[:, h, :],
                            start=(h == 0), stop=(h == 1),
                            tile_position=(0, 64), skip_group_check=True,
                        )
                    nc.scalar.copy(out=v2_all[:, j, 0:C], in_=ps_v)
                nc.vector.memset(v2_all[:, :, C : C + 1], 1.0)

        steps = 2 * BPC
        SKEW = 3
        prep_all()
        ps_qt = ctx.enter_context(tc.tile_pool(name="ps_qt", bufs=2, space="PSUM"))
        ps_st = ctx.enter_context(tc.tile_pool(name="ps_st", bufs=2, space="PSUM"))
        ps_ou = ctx.enter_context(tc.tile_pool(name="ps_ou", bufs=2, space="PSUM"))
        for st in range(0, steps + SKEW):
            if st < steps:
                phase_a(st // 2, st % 2)
            if st >= SKEW:
                pj = st - SKEW
                phase_b(pj // 2, pj % 2)

    if split:
        _split_waits(nc, limit=1)
    return nc


_NC = None


def kernel(**inputs):
    global _NC
    if _NC is None:
        _NC = _build_nc()
    q = np.ascontiguousarray(np.asarray(inputs["query"], dtype=np.float32))
    k = np.ascontiguousarray(np.asarray(inputs["key"], dtype=np.float32))
    v = np.ascontiguousarray(np.asarray(inputs["value"], dtype=np.float32))
    shared = {
        "wk": np.ascontiguousarray(np.asarray(inputs["k_proj_weight"], np.float32)),
        "wv": np.ascontiguousarray(np.asarray(inputs["v_proj_weight"], np.float32)),
        "g1": np.ascontiguousarray(np.asarray(inputs["norm1_gamma"], np.float32)),
        "b1": np.ascontiguousarray(np.asarray(inputs["norm1_beta"], np.float32)),
        "g2": np.ascontiguousarray(np.asarray(inputs["norm2_gamma"], np.float32)),
        "b2": np.ascontiguousarray(np.asarray(inputs["norm2_beta"], np.float32)),
    }
    in_maps = []
    for c in range(NCORES):
        sl = slice(c * BPC, (c + 1) * BPC)
        in_maps.append({"query": q[sl], "key": k[sl], "value": v[sl], **shared})
    res = run_bass_kernel_spmd(_NC, in_maps, core_ids=list(range(NCORES)))
    return np.concatenate([res.results[i]["out"] for i in range(NCORES)], axis=0)

